# revision 10
# baseline (speedup 1.0000x reference)
"""Trainium2 Bass kernel for nn_PixelTransformer.

Math notes (derived from the reference semantics, valid for ANY input values):
  * The transformer hidden state is built purely from positional encodings
    (x never enters it), broadcast over the batch.  The attention mixes only
    across the batch axis (head_dim=1), so with identical tokens per batch the
    softmax is uniform and the attention output equals v exactly.  Attention +
    residual therefore folds into a per-layer 5x5 linear map.
  * LayerNorm centering is a linear projection C = I - J/5, foldable into the
    preceding matmuls; LN affine params fold into the following matmuls.
  * LayerNorm is invariant to per-pixel positive scaling, and ReLU commutes
    with it.  The kernel therefore keeps the state in an UNSCALED
    representation u with g_true = (1/std) * u, tracking std per pixel.
    Biases become rank-1 accumulating matmuls (bias_row x std), and the eps
    of 1/sqrt(var+eps) becomes a rank-1 term (eps x std_prev^2) in the
    variance matmul.  Each LN is then just Square -> matmul -> Sqrt; no
    normalization is ever materialized.
  * The 16-step affine flow scan has the closed form
      z = exp(S) * x + sum_j exp(sum_{k>j} sc_k) * t_j
    computed with one triangular-matrix matmul.
  * Output is a scalar; each core emits 48 partial sums, host combines.

Sharding: the N=1024 pixels are split across 8 cores (128 each); all weights
are replicated.  Device layout keeps features on partitions and pixels on the
free dimension, so no transposes are ever needed.  The FFN (5->2048->5) and
the rank-1 bias terms run in bf16 (fp32 accumulation); the main residual
path stays fp32.
"""

import numpy as np

B, H, W = 32, 32, 32
N = H * W
L, D, FF = 8, 5, 2048
NCORES = 8
NP = N // NCORES          # pixels per core
NCHUNK = FF // 128        # 16 ff chunks of 128
NQ = NCHUNK // 4          # 4 chunks batched per PSUM bank
EPS = 1e-5

_PROG = None              # cached compiled Bass program


def _build_program():
    import concourse.bacc as bacc
    import concourse.mybir as mybir
    import concourse.tile as tile

    f32 = mybir.dt.float32
    bf16 = mybir.dt.bfloat16
    AF = mybir.ActivationFunctionType
    ALU = mybir.AluOpType
    AX = mybir.AxisListType

    nc = bacc.Bacc(name="pixel_transformer")

    smalls_d = nc.dram_tensor("smalls", [5, 80], f32, kind="ExternalInput")
    brows_d = nc.dram_tensor("brows", [1, 81], bf16, kind="ExternalInput")
    hrows_d = nc.dram_tensor("hrows", [1, 48], f32, kind="ExternalInput")
    w1b_d = [
        nc.dram_tensor(f"w1b{l}", [5, FF], bf16, kind="ExternalInput")
        for l in range(L)
    ]
    b1b_d = [
        nc.dram_tensor(f"b1b{l}", [1, FF], bf16, kind="ExternalInput")
        for l in range(L)
    ]
    w2b_d = nc.dram_tensor("w2b", [128, L * NCHUNK * D], bf16, kind="ExternalInput")
    headw_d = nc.dram_tensor("headw", [16, 66], f32, kind="ExternalInput")
    tok_d = nc.dram_tensor("tok", [5, NP], f32, kind="ExternalInput")
    xsh_d = nc.dram_tensor("xsh", [B, NP], f32, kind="ExternalInput")
    out_d = nc.dram_tensor("out", [48, 1], f32, kind="ExternalOutput")

    with tile.TileContext(nc) as tc:
        with (
            tc.tile_pool(name="consts", bufs=1) as cp,
            tc.tile_pool(name="work", bufs=2) as wp,
            tc.tile_pool(name="fsb", bufs=3) as fp,
            tc.tile_pool(name="ps", bufs=4, space="PSUM") as pp,
        ):
            # --- input DMAs, spread across SP / Pool / ACT engines ---
            w1sb = []
            b1sb = []
            for l in range(L):
                t = cp.tile([5, FF], bf16, name=f"w1sb{l}")
                tb = cp.tile([1, FF], bf16, name=f"b1sb{l}")
                eng = nc.sync if l % 2 == 0 else nc.gpsimd
                eng.dma_start(out=t, in_=w1b_d[l][:, :])
                eng.dma_start(out=tb, in_=b1b_d[l][:, :])
                w1sb.append(t)
                b1sb.append(tb)
            smalls = cp.tile([5, 80], f32)
            nc.sync.dma_start(out=smalls, in_=smalls_d[:, :])
            brows = cp.tile([1, 81], bf16)
            nc.sync.dma_start(out=brows, in_=brows_d[:, :])
            toksb = cp.tile([5, NP], f32)
            nc.sync.dma_start(out=toksb, in_=tok_d[:, :])
            w2sb = cp.tile([128, L * NCHUNK * D], bf16)
            nc.scalar.dma_start(out=w2sb, in_=w2b_d[:, :])
            headsb = cp.tile([16, 66], f32)
            nc.scalar.dma_start(out=headsb, in_=headw_d[:, :])
            hrows = cp.tile([1, 48], f32)
            nc.scalar.dma_start(out=hrows, in_=hrows_d[:, :])
            xsb = cp.tile([B, NP], f32)
            nc.scalar.dma_start(out=xsb, in_=xsh_d[:, :])

            # --- constants via memset ---
            vconst = cp.tile([5, 1], f32)       # 1/D rows for variance matmul
            nc.vector.memset(vconst, 1.0 / D)
            ones16 = cp.tile([16, 1], f32)      # column-sum over 16 partitions
            nc.vector.memset(ones16, 1.0)
            ones_b16 = cp.tile([1, 16], f32)    # broadcast 1 -> 16 partitions
            nc.vector.memset(ones_b16, 1.0)
            ones_b32 = cp.tile([1, B], f32)     # broadcast 1 -> 32 partitions
            nc.vector.memset(ones_b32, 1.0)
            std0b = cp.tile([1, NP], bf16)      # std before layer 0 == 1
            nc.vector.memset(std0b, 1.0)
            psv0b = cp.tile([1, NP], bf16)      # var+eps proxy before layer 0
            nc.vector.memset(psv0b, 1.0)

            def ln_v2(ps_y, psvb_prev, idx, fp32_std=False, want_psvb=True):
                """LN in the scaled representation: unscaled centered PSUM
                ps_y -> (u [5,NP] f32, std bf16 (or f32), psv bf16)."""
                sq = wp.tile([5, NP], f32, tag="sq", name=f"sq{idx}")
                nc.scalar.activation(out=sq, in_=ps_y, func=AF.Square)
                psv = pp.tile([1, NP], f32, tag="psmall", name=f"psv{idx}")
                nc.tensor.matmul(
                    psv, brows[:, 80:81], psvb_prev, start=True, stop=False
                )
                nc.tensor.matmul(psv, vconst, sq, start=False, stop=True)
                u = wp.tile([5, NP], f32, tag="u", bufs=3, name=f"u{idx}")
                nc.vector.tensor_copy(out=u, in_=ps_y)
                stdt = f32 if fp32_std else bf16
                stdb = wp.tile([1, NP], stdt, tag="stdb", bufs=3,
                               name=f"stdb{idx}")
                nc.scalar.activation(out=stdb, in_=psv, func=AF.Sqrt)
                psvb = None
                if want_psvb:
                    psvb = wp.tile([1, NP], bf16, tag="psvb", bufs=3,
                                   name=f"psvb{idx}")
                    nc.vector.tensor_copy(out=psvb, in_=psv)
                return u, stdb, psvb

            u_prev, stdb_prev, psvb_prev = toksb, std0b, psv0b
            for l in range(L):
                # attention (folded) + residual + LN1 centering
                psy = pp.tile([D, NP], f32, tag="psmall", name=f"psy{l}")
                nc.tensor.matmul(
                    psy, smalls[:, 5 * l:5 * l + 5], u_prev,
                    start=True, stop=False,
                )
                nc.tensor.matmul(
                    psy, brows[:, 5 * l:5 * l + 5], stdb_prev,
                    start=False, stop=True,
                )
                u1, std1b, psvb1 = ln_v2(psy, psvb_prev, f"a{l}")
                u1b = wp.tile([5, NP], bf16, tag="ub", name=f"ub{l}")
                nc.vector.tensor_copy(out=u1b, in_=psy)

                # FFN in bf16; psum_y2 accumulates the centered layer output
                psy2 = pp.tile([D, NP], f32, tag="pacc", bufs=2, name=f"psy2{l}")
                nc.tensor.matmul(
                    psy2, smalls[:, 40 + 5 * l:45 + 5 * l], u1,
                    start=True, stop=False,
                )
                nc.tensor.matmul(
                    psy2, brows[:, 40 + 5 * l:45 + 5 * l], std1b,
                    start=False, stop=False,
                )
                for q in range(NQ):
                    psf = pp.tile([128, 512], f32, tag="pf", bufs=2,
                                  name=f"psf{l}_{q}")
                    for c4 in range(4):
                        c = 4 * q + c4
                        nc.tensor.matmul(
                            psf[:, 128 * c4:128 * (c4 + 1)],
                            w1sb[l][:, 128 * c:128 * (c + 1)],
                            u1b, start=True, stop=False,
                        )
                        nc.tensor.matmul(
                            psf[:, 128 * c4:128 * (c4 + 1)],
                            b1sb[l][:, 128 * c:128 * (c + 1)],
                            std1b, start=False, stop=True,
                        )
                    fq = fp.tile([128, 512], bf16, tag="f", name=f"f{l}_{q}")
                    if q % 2 == 0:
                        nc.scalar.activation(out=fq, in_=psf, func=AF.Relu)
                    else:
                        nc.vector.tensor_scalar(
                            out=fq, in0=psf, scalar1=0.0, scalar2=None,
                            op0=ALU.max,
                        )
                    for c4 in range(4):
                        c = 4 * q + c4
                        nc.tensor.matmul(
                            psy2,
                            w2sb[:, (NCHUNK * l + c) * D:(NCHUNK * l + c + 1) * D],
                            fq[:, 128 * c4:128 * (c4 + 1)],
                            start=False,
                            stop=(c == NCHUNK - 1),
                        )
                u_prev, stdb_prev, psvb_prev = ln_v2(
                    psy2, psvb1, f"b{l}", fp32_std=(l == L - 1),
                    want_psvb=(l < L - 1),
                )
            std8 = stdb_prev

            # ---- head ----
            # hid_u = relu(f0w1'@u8 + f0b1' x std8)
            psh = pp.tile([16, NP], f32, tag="psmall")
            nc.tensor.matmul(psh, headsb[0:5, 0:16], u_prev, start=True, stop=False)
            nc.tensor.matmul(psh, hrows[:, 0:16], std8, start=False, stop=True)
            hid = wp.tile([16, NP], f32, tag="sq")
            nc.scalar.activation(out=hid, in_=psh, func=AF.Relu)

            # st halves, still scaled by 1/std8
            psss = pp.tile([16, NP], f32, tag="psmall")
            nc.tensor.matmul(psss, headsb[:, 16:32], hid, start=True, stop=False)
            nc.tensor.matmul(psss, hrows[:, 16:32], std8, start=False, stop=True)
            psst = pp.tile([16, NP], f32, tag="pf", bufs=2)
            nc.tensor.matmul(psst, headsb[:, 32:48], hid, start=True, stop=False)
            nc.tensor.matmul(psst, hrows[:, 32:48], std8, start=False, stop=True)

            # materialize absolute st: r8 = 1/std8 broadcast to 16 partitions
            rec8 = wp.tile([1, NP], f32, tag="rec8")
            nc.vector.reciprocal(out=rec8, in_=std8)
            psbc = pp.tile([16, NP], f32, tag="pacc", bufs=2)
            nc.tensor.matmul(psbc, ones_b16, rec8, start=True, stop=True)
            r8bc = wp.tile([16, NP], f32, tag="r8bc")
            nc.vector.tensor_copy(out=r8bc, in_=psbc)
            s_sb = wp.tile([16, NP], f32, tag="u", bufs=3)
            nc.vector.tensor_mul(out=s_sb, in0=psss, in1=r8bc)
            t_sb = wp.tile([16, NP], f32, tag="ub")
            nc.vector.tensor_mul(out=t_sb, in0=psst, in1=r8bc)

            ssum = wp.tile([16, 1], f32, tag="ssum")
            nc.vector.reduce_sum(out=ssum, in_=s_sb, axis=AX.X)

            sf = wp.tile([16, 1], f32, tag="sf")
            nc.scalar.activation(out=sf, in_=headsb[0:16, 48:49], func=AF.Exp)
            rsf = wp.tile([16, 1], f32, tag="rsf")
            nc.vector.reciprocal(out=rsf, in_=sf)
            sc1 = wp.tile([16, NP], f32, tag="sc1")
            nc.vector.tensor_scalar(
                out=sc1, in0=s_sb, scalar1=rsf, scalar2=None, op0=ALU.mult
            )
            th = wp.tile([16, NP], f32, tag="th")
            nc.scalar.activation(out=th, in_=sc1, func=AF.Tanh)
            sc = wp.tile([16, NP], f32, tag="sc")
            nc.vector.tensor_scalar(
                out=sc, in0=th, scalar1=sf, scalar2=None, op0=ALU.mult
            )

            # flow closed form
            psD = pp.tile([16, NP], f32, tag="psmall")
            nc.tensor.matmul(psD, headsb[0:16, 49:65], sc, start=True, stop=True)
            psS = pp.tile([1, NP], f32, tag="pf", bufs=2)
            nc.tensor.matmul(psS, ones16, sc, start=True, stop=True)
            wexp = wp.tile([16, NP], f32, tag="sc1")
            nc.scalar.activation(out=wexp, in_=psD, func=AF.Exp)
            eS = wp.tile([1, NP], f32, tag="eS")
            nc.scalar.activation(out=eS, in_=psS, func=AF.Exp)
            wt = wp.tile([16, NP], f32, tag="th")
            nc.vector.tensor_mul(out=wt, in0=wexp, in1=t_sb)
            psc1 = pp.tile([1, NP], f32, tag="psmall")
            nc.tensor.matmul(psc1, ones16, wt, start=True, stop=True)
            c1sb = wp.tile([1, NP], f32, tag="c1")
            nc.vector.tensor_copy(out=c1sb, in_=psc1)

            pseb = pp.tile([B, NP], f32, tag="pacc", bufs=2)
            nc.tensor.matmul(pseb, ones_b32, eS, start=True, stop=True)
            pscb = pp.tile([B, NP], f32, tag="psmall")
            nc.tensor.matmul(pscb, ones_b32, c1sb, start=True, stop=True)

            zt = wp.tile([B, NP], f32, tag="zt")
            nc.vector.tensor_mul(out=zt, in0=xsb, in1=pseb)
            z = wp.tile([B, NP], f32, tag="z")
            nc.vector.tensor_add(out=z, in0=zt, in1=pscb)
            zsq = wp.tile([B, NP], f32, tag="zt2")
            nc.scalar.activation(out=zsq, in_=z, func=AF.Square)
            zrow = wp.tile([B, 1], f32, tag="zrow")
            nc.vector.reduce_sum(out=zrow, in_=zsq, axis=AX.X)

            nc.sync.dma_start(out=out_d[0:16, :], in_=ssum)
            nc.sync.dma_start(out=out_d[16:48, :], in_=zrow)

    nc.finalize()
    return nc


def _fold_inputs(inp):
    """Host-side weight folding (float64 for precision, cast at the end)."""
    import ml_dtypes

    C = np.eye(D) - np.ones((D, D)) / D
    g = lambda k: np.asarray(inp[k], dtype=np.float64)
    wqkv, bqkv, wo, bo = g("wqkv"), g("bqkv"), g("wo"), g("bo")
    w1, b1, w2, b2 = g("w1"), g("b1"), g("w2"), g("b2")
    ln1w, ln1b, ln2w, ln2b = g("ln1w"), g("ln1b"), g("ln2w"), g("ln2b")

    smalls = np.zeros((5, 80), np.float64)
    brows = np.zeros((1, 81), np.float64)
    w1a = []
    b1b = np.zeros((L, 1, FF), np.float64)
    w2b = np.zeros((128, L * NCHUNK * D), np.float64)
    for l in range(L):
        Dl = np.diag(ln2w[l - 1]) if l > 0 else np.eye(D)
        el = ln2b[l - 1] if l > 0 else np.zeros(D)
        wv = wqkv[l][2 * D:3 * D, :]
        bv = bqkv[l][2 * D:3 * D]
        A0 = np.eye(D) + wo[l] @ wv
        c_attn = wo[l] @ bv + bo[l]
        smalls[:, 5 * l:5 * l + 5] = (C @ A0 @ Dl).T
        brows[0, 5 * l:5 * l + 5] = C @ (A0 @ el + c_attn)
        smalls[:, 40 + 5 * l:45 + 5 * l] = (C @ np.diag(ln1w[l])).T
        brows[0, 40 + 5 * l:45 + 5 * l] = C @ (ln1b[l] + b2[l])
        w1a.append((w1[l] * ln1w[l][None, :]).T)       # [5, FF]
        b1b[l, 0, :] = b1[l] + w1[l] @ ln1b[l]
        w2full = (C @ w2[l]).T                         # [2048, 5]
        for c in range(NCHUNK):
            w2b[:, (NCHUNK * l + c) * D:(NCHUNK * l + c + 1) * D] = (
                w2full[128 * c:128 * (c + 1), :]
            )
    brows[0, 80] = EPS

    f0w1, f0b1 = g("f0w1"), g("f0b1")
    f0w2, f0b2 = g("f0w2"), g("f0b2")
    D8 = np.diag(ln2w[L - 1])
    e8 = ln2b[L - 1]
    headw = np.zeros((16, 66), np.float64)
    headw[0:5, 0:16] = (f0w1 @ D8).T
    headw[0:16, 16:32] = f0w2.T[:, 0:16]
    headw[0:16, 32:48] = f0w2.T[:, 16:32]
    headw[0:16, 48] = float(np.asarray(inp["sfac"])[0])
    for j in range(16):
        headw[j + 1:16, 49 + j] = 1.0                 # sum_{k>j}
    hrows = np.zeros((1, 48), np.float64)
    hrows[0, 0:16] = f0b1 + f0w1 @ e8
    hrows[0, 16:32] = f0b2[0:16]
    hrows[0, 32:48] = f0b2[16:32]

    # positional tokens, exactly as the reference builds them (fp32 ops)
    xs = (np.arange(W, dtype=np.float32) / np.float32(1e4)).astype(np.float32)
    ys = (np.arange(H, dtype=np.float32) / np.float32(1e4)).astype(np.float32)
    sinx = np.broadcast_to(np.sin(xs)[None, :], (H, W)).reshape(N)
    cosx = np.broadcast_to(np.cos(xs)[None, :], (H, W)).reshape(N)
    siny = np.broadcast_to(np.sin(ys)[:, None], (H, W)).reshape(N)
    cosy = np.broadcast_to(np.cos(ys)[:, None], (H, W)).reshape(N)
    tok = np.stack(
        [-np.ones(N, np.float32), sinx, cosx, siny, cosy], axis=0
    )                                                  # [5, N]
    xflat = np.asarray(inp["x"], dtype=np.float32)[:, 0].reshape(B, N)

    out = {
        "smalls": smalls.astype(np.float32),
        "brows": brows.astype(ml_dtypes.bfloat16),
        "hrows": hrows.astype(np.float32),
        "w2b": w2b.astype(ml_dtypes.bfloat16),
        "headw": headw.astype(np.float32),
        "tok": tok.astype(np.float32),
        "xsh": xflat,
    }
    for l in range(L):
        out[f"w1b{l}"] = w1a[l].astype(ml_dtypes.bfloat16)
        out[f"b1b{l}"] = b1b[l].astype(ml_dtypes.bfloat16)
    return out


def get_program():
    global _PROG
    if _PROG is None:
        _PROG = _build_program()
    return _PROG


def make_in_maps(inputs):
    arrs = _fold_inputs(inputs)
    shared_keys = (
        ["smalls", "brows", "hrows", "w2b", "headw"]
        + [f"w1b{l}" for l in range(L)]
        + [f"b1b{l}" for l in range(L)]
    )
    shared = {k: arrs[k] for k in shared_keys}
    in_maps = []
    for core in range(NCORES):
        sl = slice(core * NP, (core + 1) * NP)
        m = dict(shared)
        m["tok"] = np.ascontiguousarray(arrs["tok"][:, sl])
        m["xsh"] = np.ascontiguousarray(arrs["xsh"][:, sl])
        in_maps.append(m)
    return in_maps


def combine_outputs(outs):
    """outs: list of per-core [48, 1] arrays -> scalar float32."""
    s_tot = 0.0
    q_tot = 0.0
    for o in outs:
        o = np.asarray(o, dtype=np.float64).reshape(48)
        s_tot += o[0:16].sum()
        q_tot += o[16:48].sum()
    sldj = B * s_tot - 0.5 * q_tot - B * N * 0.5 * np.log(2.0 * np.pi)
    return np.array(-sldj, dtype=np.float32)


def kernel(**inputs):
    from concourse.bass_utils import run_bass_kernel_spmd

    nc = get_program()
    in_maps = make_in_maps(inputs)
    res = run_bass_kernel_spmd(nc, in_maps, core_ids=list(range(NCORES)))
    return combine_outputs([r["out"] for r in res.results])


# revision 14
# speedup vs baseline: 1.0687x; 1.0687x over previous
"""Trainium2 Bass kernel for nn_PixelTransformer.

Math notes (derived from the reference semantics, valid for ANY input values):
  * The transformer hidden state is built purely from positional encodings
    (x never enters it), broadcast over the batch.  The attention mixes only
    across the batch axis (head_dim=1), so with identical tokens per batch the
    softmax is uniform and the attention output equals v exactly.  Attention +
    residual therefore folds into a per-layer 5x5 linear map.
  * LayerNorm centering is a linear projection C = I - J/5, foldable into the
    preceding matmuls; LN affine params fold into the following matmuls.
  * LayerNorm is invariant to per-pixel positive scaling, and ReLU commutes
    with it.  The kernel therefore keeps the state in an UNSCALED
    representation u with g_true = (1/std) * u, tracking std per pixel.
    Biases become rank-1 accumulating matmuls (bias_row x std), and the eps
    of 1/sqrt(var+eps) becomes a rank-1 term (eps x std_prev^2) in the
    variance matmul.  Each LN is then just Square -> matmul -> Sqrt; no
    normalization is ever materialized.
  * The 16-step affine flow scan has the closed form
      z = exp(S) * x + sum_j exp(sum_{k>j} sc_k) * t_j
    computed with one triangular-matrix matmul.
  * Output is a scalar; each core emits 48 partial sums, host combines.

Sharding: the N=1024 pixels are split across 8 cores (128 each); all weights
are replicated.  Device layout keeps features on partitions and pixels on the
free dimension, so no transposes are ever needed.  The FFN (5->2048->5) and
the rank-1 bias terms run in bf16 (fp32 accumulation); the main residual
path stays fp32.
"""

import numpy as np

B, H, W = 32, 32, 32
N = H * W
L, D, FF = 8, 5, 2048
NCORES = 8
NP = N // NCORES          # pixels per core
NCHUNK = FF // 128        # 16 ff chunks of 128
NQ = NCHUNK // 4          # 4 chunks batched per PSUM bank
EPS = 1e-5

_PROG = None              # cached compiled Bass program


def _build_program():
    import concourse.bacc as bacc
    import concourse.mybir as mybir
    import concourse.tile as tile

    f32 = mybir.dt.float32
    bf16 = mybir.dt.bfloat16
    AF = mybir.ActivationFunctionType
    ALU = mybir.AluOpType
    AX = mybir.AxisListType

    nc = bacc.Bacc(name="pixel_transformer")

    smalls_d = nc.dram_tensor("smalls", [5, 80], f32, kind="ExternalInput")
    brows_d = nc.dram_tensor("brows", [1, 81], bf16, kind="ExternalInput")
    hrows_d = nc.dram_tensor("hrows", [1, 48], f32, kind="ExternalInput")
    w1b_d = [
        nc.dram_tensor(f"w1b{l}", [5, FF], bf16, kind="ExternalInput")
        for l in range(L)
    ]
    b1b_d = [
        nc.dram_tensor(f"b1b{l}", [1, FF], bf16, kind="ExternalInput")
        for l in range(L)
    ]
    w2b_d = nc.dram_tensor("w2b", [128, L * NCHUNK * D], bf16, kind="ExternalInput")
    headw_d = nc.dram_tensor("headw", [16, 66], f32, kind="ExternalInput")
    tok_d = nc.dram_tensor("tok", [5, NP], f32, kind="ExternalInput")
    xsh_d = nc.dram_tensor("xsh", [B, NP], f32, kind="ExternalInput")
    out_d = nc.dram_tensor("out", [48, 1], f32, kind="ExternalOutput")

    with tile.TileContext(nc) as tc:
        with (
            tc.tile_pool(name="consts", bufs=1) as cp,
            tc.tile_pool(name="work", bufs=2) as wp,
            tc.tile_pool(name="fsb", bufs=3) as fp,
            tc.tile_pool(name="ps", bufs=2, space="PSUM") as pp,
        ):
            # --- input DMAs, spread across SP / Pool / ACT engines ---
            w1sb = []
            b1sb = []
            for l in range(L):
                t = cp.tile([5, FF], bf16, name=f"w1sb{l}")
                tb = cp.tile([1, FF], bf16, name=f"b1sb{l}")
                eng = nc.sync if l % 2 == 0 else nc.gpsimd
                eng.dma_start(out=t, in_=w1b_d[l][:, :])
                eng.dma_start(out=tb, in_=b1b_d[l][:, :])
                w1sb.append(t)
                b1sb.append(tb)
            smalls = cp.tile([5, 80], f32)
            nc.sync.dma_start(out=smalls, in_=smalls_d[:, :])
            brows = cp.tile([1, 81], bf16)
            nc.sync.dma_start(out=brows, in_=brows_d[:, :])
            toksb = cp.tile([5, NP], f32)
            nc.sync.dma_start(out=toksb, in_=tok_d[:, :])
            w2sb = cp.tile([128, L * NCHUNK * D], bf16)
            nc.scalar.dma_start(out=w2sb, in_=w2b_d[:, :])
            headsb = cp.tile([16, 66], f32)
            nc.scalar.dma_start(out=headsb, in_=headw_d[:, :])
            hrows = cp.tile([1, 48], f32)
            nc.scalar.dma_start(out=hrows, in_=hrows_d[:, :])
            xsb = cp.tile([B, NP], f32)
            nc.scalar.dma_start(out=xsb, in_=xsh_d[:, :])

            # --- constants via memset ---
            vconst = cp.tile([5, 1], f32)       # 1/D rows for variance matmul
            nc.vector.memset(vconst, 1.0 / D)
            ones16 = cp.tile([16, 1], f32)      # column-sum over 16 partitions
            nc.vector.memset(ones16, 1.0)
            ones_b16 = cp.tile([1, 16], f32)    # broadcast 1 -> 16 partitions
            nc.vector.memset(ones_b16, 1.0)
            ones_b32 = cp.tile([1, B], f32)     # broadcast 1 -> 32 partitions
            nc.vector.memset(ones_b32, 1.0)
            std0b = cp.tile([1, NP], bf16)      # std before layer 0 == 1
            nc.vector.memset(std0b, 1.0)
            psv0b = cp.tile([1, NP], bf16)      # var+eps proxy before layer 0
            nc.vector.memset(psv0b, 1.0)

            def ln_v2(ps_y, psvb_prev, idx, fp32_std=False, want_psvb=True):
                """LN in the scaled representation: unscaled centered PSUM
                ps_y -> (u [5,NP] f32, std bf16 (or f32), psv bf16)."""
                sq = wp.tile([5, NP], f32, tag="sq", name=f"sq{idx}")
                nc.scalar.activation(out=sq, in_=ps_y, func=AF.Square)
                psv = pp.tile([1, NP], f32, tag="psmall", name=f"psv{idx}")
                nc.tensor.matmul(
                    psv, brows[:, 80:81], psvb_prev, start=True, stop=False
                )
                nc.tensor.matmul(psv, vconst, sq, start=False, stop=True)
                u = wp.tile([5, NP], f32, tag="u", bufs=3, name=f"u{idx}")
                nc.vector.tensor_copy(out=u, in_=ps_y)
                stdt = f32 if fp32_std else bf16
                stdb = wp.tile([1, NP], stdt, tag="stdb", bufs=3,
                               name=f"stdb{idx}")
                nc.scalar.activation(out=stdb, in_=psv, func=AF.Sqrt)
                psvb = None
                if want_psvb:
                    psvb = wp.tile([1, NP], bf16, tag="psvb", bufs=3,
                                   name=f"psvb{idx}")
                    nc.vector.tensor_copy(out=psvb, in_=psv)
                return u, stdb, psvb

            u_prev, stdb_prev, psvb_prev = toksb, std0b, psv0b
            for l in range(L):
                # attention (folded) + residual + LN1 centering
                psy = pp.tile([D, NP], f32, tag="psmall", name=f"psy{l}")
                nc.tensor.matmul(
                    psy, smalls[:, 5 * l:5 * l + 5], u_prev,
                    start=True, stop=False,
                )
                nc.tensor.matmul(
                    psy, brows[:, 5 * l:5 * l + 5], stdb_prev,
                    start=False, stop=True,
                )
                u1, std1b, psvb1 = ln_v2(psy, psvb_prev, f"a{l}")
                u1b = wp.tile([5, NP], bf16, tag="ub", name=f"ub{l}")
                nc.vector.tensor_copy(out=u1b, in_=psy)

                # FFN in bf16; psum_y2 accumulates the centered layer output
                psy2 = pp.tile([D, NP], f32, tag="pacc", bufs=2, name=f"psy2{l}")
                nc.tensor.matmul(
                    psy2, smalls[:, 40 + 5 * l:45 + 5 * l], u1,
                    start=True, stop=False,
                )
                # all mm1 mains first (only need u1b, ready early); the
                # rank-1 bias matmuls wait on sqrt -> keep them behind so
                # they don't block the mains in PE program order
                psfs = []
                for q in range(NQ):
                    psf = pp.tile([128, 512], f32, tag="pf", bufs=4,
                                  name=f"psf{l}_{q}")
                    psfs.append(psf)
                    for c4 in range(4):
                        c = 4 * q + c4
                        nc.tensor.matmul(
                            psf[:, 128 * c4:128 * (c4 + 1)],
                            w1sb[l][:, 128 * c:128 * (c + 1)],
                            u1b, start=(c4 == 0), stop=False,
                        )
                for q in range(NQ):
                    for c4 in range(4):
                        c = 4 * q + c4
                        nc.tensor.matmul(
                            psfs[q][:, 128 * c4:128 * (c4 + 1)],
                            b1sb[l][:, 128 * c:128 * (c + 1)],
                            std1b, start=False, stop=(c4 == 3),
                        )
                nc.tensor.matmul(
                    psy2, brows[:, 40 + 5 * l:45 + 5 * l], std1b,
                    start=False, stop=False,
                )
                for q in range(NQ):
                    fq = fp.tile([128, 512], bf16, tag="f", name=f"f{l}_{q}")
                    if q % 2 == 0:
                        nc.scalar.activation(out=fq, in_=psfs[q], func=AF.Relu)
                    else:
                        nc.vector.tensor_scalar(
                            out=fq, in0=psfs[q], scalar1=0.0, scalar2=None,
                            op0=ALU.max,
                        )
                    for c4 in range(4):
                        c = 4 * q + c4
                        nc.tensor.matmul(
                            psy2,
                            w2sb[:, (NCHUNK * l + c) * D:(NCHUNK * l + c + 1) * D],
                            fq[:, 128 * c4:128 * (c4 + 1)],
                            start=False,
                            stop=(c == NCHUNK - 1),
                        )
                u_prev, stdb_prev, psvb_prev = ln_v2(
                    psy2, psvb1, f"b{l}", fp32_std=(l == L - 1),
                    want_psvb=(l < L - 1),
                )
            std8 = stdb_prev

            # ---- head ----
            # hid_u = relu(f0w1'@u8 + f0b1' x std8)
            psh = pp.tile([16, NP], f32, tag="psmall")
            nc.tensor.matmul(psh, headsb[0:5, 0:16], u_prev, start=True, stop=False)
            nc.tensor.matmul(psh, hrows[:, 0:16], std8, start=False, stop=True)
            hid = wp.tile([16, NP], f32, tag="sq")
            nc.scalar.activation(out=hid, in_=psh, func=AF.Relu)

            # st halves, still scaled by 1/std8
            psss = pp.tile([16, NP], f32, tag="psmall")
            nc.tensor.matmul(psss, headsb[:, 16:32], hid, start=True, stop=False)
            nc.tensor.matmul(psss, hrows[:, 16:32], std8, start=False, stop=True)
            psst = pp.tile([16, NP], f32, tag="pf", bufs=4)
            nc.tensor.matmul(psst, headsb[:, 32:48], hid, start=True, stop=False)
            nc.tensor.matmul(psst, hrows[:, 32:48], std8, start=False, stop=True)

            # materialize absolute st: r8 = 1/std8 broadcast to 16 partitions
            rec8 = wp.tile([1, NP], f32, tag="rec8")
            nc.vector.reciprocal(out=rec8, in_=std8)
            psbc = pp.tile([16, NP], f32, tag="pacc", bufs=2)
            nc.tensor.matmul(psbc, ones_b16, rec8, start=True, stop=True)
            r8bc = wp.tile([16, NP], f32, tag="r8bc")
            nc.vector.tensor_copy(out=r8bc, in_=psbc)
            s_sb = wp.tile([16, NP], f32, tag="u", bufs=3)
            nc.vector.tensor_mul(out=s_sb, in0=psss, in1=r8bc)
            t_sb = wp.tile([16, NP], f32, tag="ub")
            nc.vector.tensor_mul(out=t_sb, in0=psst, in1=r8bc)

            ssum = wp.tile([16, 1], f32, tag="ssum")
            nc.vector.reduce_sum(out=ssum, in_=s_sb, axis=AX.X)

            sf = wp.tile([16, 1], f32, tag="sf")
            nc.scalar.activation(out=sf, in_=headsb[0:16, 48:49], func=AF.Exp)
            rsf = wp.tile([16, 1], f32, tag="rsf")
            nc.vector.reciprocal(out=rsf, in_=sf)
            sc1 = wp.tile([16, NP], f32, tag="sc1")
            nc.vector.tensor_scalar(
                out=sc1, in0=s_sb, scalar1=rsf, scalar2=None, op0=ALU.mult
            )
            th = wp.tile([16, NP], f32, tag="th")
            nc.scalar.activation(out=th, in_=sc1, func=AF.Tanh)
            sc = wp.tile([16, NP], f32, tag="sc")
            nc.vector.tensor_scalar(
                out=sc, in0=th, scalar1=sf, scalar2=None, op0=ALU.mult
            )

            # flow closed form
            psD = pp.tile([16, NP], f32, tag="psmall")
            nc.tensor.matmul(psD, headsb[0:16, 49:65], sc, start=True, stop=True)
            psS = pp.tile([1, NP], f32, tag="pf", bufs=4)
            nc.tensor.matmul(psS, ones16, sc, start=True, stop=True)
            wexp = wp.tile([16, NP], f32, tag="sc1")
            nc.scalar.activation(out=wexp, in_=psD, func=AF.Exp)
            eS = wp.tile([1, NP], f32, tag="eS")
            nc.scalar.activation(out=eS, in_=psS, func=AF.Exp)
            wt = wp.tile([16, NP], f32, tag="th")
            nc.vector.tensor_mul(out=wt, in0=wexp, in1=t_sb)
            psc1 = pp.tile([1, NP], f32, tag="psmall")
            nc.tensor.matmul(psc1, ones16, wt, start=True, stop=True)
            c1sb = wp.tile([1, NP], f32, tag="c1")
            nc.vector.tensor_copy(out=c1sb, in_=psc1)

            pseb = pp.tile([B, NP], f32, tag="pacc", bufs=2)
            nc.tensor.matmul(pseb, ones_b32, eS, start=True, stop=True)
            pscb = pp.tile([B, NP], f32, tag="psmall")
            nc.tensor.matmul(pscb, ones_b32, c1sb, start=True, stop=True)

            zt = wp.tile([B, NP], f32, tag="zt")
            nc.vector.tensor_mul(out=zt, in0=xsb, in1=pseb)
            z = wp.tile([B, NP], f32, tag="z")
            nc.vector.tensor_add(out=z, in0=zt, in1=pscb)
            zsq = wp.tile([B, NP], f32, tag="zt2")
            nc.scalar.activation(out=zsq, in_=z, func=AF.Square)
            zrow = wp.tile([B, 1], f32, tag="zrow")
            nc.vector.reduce_sum(out=zrow, in_=zsq, axis=AX.X)

            nc.sync.dma_start(out=out_d[0:16, :], in_=ssum)
            nc.sync.dma_start(out=out_d[16:48, :], in_=zrow)

    nc.finalize()
    return nc


def _fold_inputs(inp):
    """Host-side weight folding (float64 for precision, cast at the end)."""
    import ml_dtypes

    C = np.eye(D) - np.ones((D, D)) / D
    g = lambda k: np.asarray(inp[k], dtype=np.float64)
    wqkv, bqkv, wo, bo = g("wqkv"), g("bqkv"), g("wo"), g("bo")
    w1, b1, w2, b2 = g("w1"), g("b1"), g("w2"), g("b2")
    ln1w, ln1b, ln2w, ln2b = g("ln1w"), g("ln1b"), g("ln2w"), g("ln2b")

    smalls = np.zeros((5, 80), np.float64)
    brows = np.zeros((1, 81), np.float64)
    w1a = []
    b1b = np.zeros((L, 1, FF), np.float64)
    w2b = np.zeros((128, L * NCHUNK * D), np.float64)
    for l in range(L):
        Dl = np.diag(ln2w[l - 1]) if l > 0 else np.eye(D)
        el = ln2b[l - 1] if l > 0 else np.zeros(D)
        wv = wqkv[l][2 * D:3 * D, :]
        bv = bqkv[l][2 * D:3 * D]
        A0 = np.eye(D) + wo[l] @ wv
        c_attn = wo[l] @ bv + bo[l]
        smalls[:, 5 * l:5 * l + 5] = (C @ A0 @ Dl).T
        brows[0, 5 * l:5 * l + 5] = C @ (A0 @ el + c_attn)
        smalls[:, 40 + 5 * l:45 + 5 * l] = (C @ np.diag(ln1w[l])).T
        brows[0, 40 + 5 * l:45 + 5 * l] = C @ (ln1b[l] + b2[l])
        w1a.append((w1[l] * ln1w[l][None, :]).T)       # [5, FF]
        b1b[l, 0, :] = b1[l] + w1[l] @ ln1b[l]
        w2full = (C @ w2[l]).T                         # [2048, 5]
        for c in range(NCHUNK):
            w2b[:, (NCHUNK * l + c) * D:(NCHUNK * l + c + 1) * D] = (
                w2full[128 * c:128 * (c + 1), :]
            )
    brows[0, 80] = EPS

    f0w1, f0b1 = g("f0w1"), g("f0b1")
    f0w2, f0b2 = g("f0w2"), g("f0b2")
    D8 = np.diag(ln2w[L - 1])
    e8 = ln2b[L - 1]
    headw = np.zeros((16, 66), np.float64)
    headw[0:5, 0:16] = (f0w1 @ D8).T
    headw[0:16, 16:32] = f0w2.T[:, 0:16]
    headw[0:16, 32:48] = f0w2.T[:, 16:32]
    headw[0:16, 48] = float(np.asarray(inp["sfac"])[0])
    for j in range(16):
        headw[j + 1:16, 49 + j] = 1.0                 # sum_{k>j}
    hrows = np.zeros((1, 48), np.float64)
    hrows[0, 0:16] = f0b1 + f0w1 @ e8
    hrows[0, 16:32] = f0b2[0:16]
    hrows[0, 32:48] = f0b2[16:32]

    # positional tokens, exactly as the reference builds them (fp32 ops)
    xs = (np.arange(W, dtype=np.float32) / np.float32(1e4)).astype(np.float32)
    ys = (np.arange(H, dtype=np.float32) / np.float32(1e4)).astype(np.float32)
    sinx = np.broadcast_to(np.sin(xs)[None, :], (H, W)).reshape(N)
    cosx = np.broadcast_to(np.cos(xs)[None, :], (H, W)).reshape(N)
    siny = np.broadcast_to(np.sin(ys)[:, None], (H, W)).reshape(N)
    cosy = np.broadcast_to(np.cos(ys)[:, None], (H, W)).reshape(N)
    tok = np.stack(
        [-np.ones(N, np.float32), sinx, cosx, siny, cosy], axis=0
    )                                                  # [5, N]
    xflat = np.asarray(inp["x"], dtype=np.float32)[:, 0].reshape(B, N)

    out = {
        "smalls": smalls.astype(np.float32),
        "brows": brows.astype(ml_dtypes.bfloat16),
        "hrows": hrows.astype(np.float32),
        "w2b": w2b.astype(ml_dtypes.bfloat16),
        "headw": headw.astype(np.float32),
        "tok": tok.astype(np.float32),
        "xsh": xflat,
    }
    for l in range(L):
        out[f"w1b{l}"] = w1a[l].astype(ml_dtypes.bfloat16)
        out[f"b1b{l}"] = b1b[l].astype(ml_dtypes.bfloat16)
    return out


def get_program():
    global _PROG
    if _PROG is None:
        _PROG = _build_program()
    return _PROG


def make_in_maps(inputs):
    arrs = _fold_inputs(inputs)
    shared_keys = (
        ["smalls", "brows", "hrows", "w2b", "headw"]
        + [f"w1b{l}" for l in range(L)]
        + [f"b1b{l}" for l in range(L)]
    )
    shared = {k: arrs[k] for k in shared_keys}
    in_maps = []
    for core in range(NCORES):
        sl = slice(core * NP, (core + 1) * NP)
        m = dict(shared)
        m["tok"] = np.ascontiguousarray(arrs["tok"][:, sl])
        m["xsh"] = np.ascontiguousarray(arrs["xsh"][:, sl])
        in_maps.append(m)
    return in_maps


def combine_outputs(outs):
    """outs: list of per-core [48, 1] arrays -> scalar float32."""
    s_tot = 0.0
    q_tot = 0.0
    for o in outs:
        o = np.asarray(o, dtype=np.float64).reshape(48)
        s_tot += o[0:16].sum()
        q_tot += o[16:48].sum()
    sldj = B * s_tot - 0.5 * q_tot - B * N * 0.5 * np.log(2.0 * np.pi)
    return np.array(-sldj, dtype=np.float32)


def kernel(**inputs):
    from concourse.bass_utils import run_bass_kernel_spmd

    nc = get_program()
    in_maps = make_in_maps(inputs)
    res = run_bass_kernel_spmd(nc, in_maps, core_ids=list(range(NCORES)))
    return combine_outputs([r["out"] for r in res.results])


# revision 15
# speedup vs baseline: 1.0957x; 1.0253x over previous
"""Trainium2 Bass kernel for nn_PixelTransformer.

Math notes (derived from the reference semantics, valid for ANY input values):
  * The transformer hidden state is built purely from positional encodings
    (x never enters it), broadcast over the batch.  The attention mixes only
    across the batch axis (head_dim=1), so with identical tokens per batch the
    softmax is uniform and the attention output equals v exactly.  Attention +
    residual therefore folds into a per-layer 5x5 linear map.
  * LayerNorm centering is a linear projection C = I - J/5, foldable into the
    preceding matmuls; LN affine params fold into the following matmuls.
  * LayerNorm is invariant to per-pixel positive scaling, and ReLU commutes
    with it.  The kernel therefore keeps the state in an UNSCALED
    representation u with g_true = (1/std) * u, tracking std per pixel.
    Biases become rank-1 accumulating matmuls (bias_row x std), and the eps
    of 1/sqrt(var+eps) becomes a rank-1 term (eps x std_prev^2) in the
    variance matmul.  Each LN is then just Square -> matmul -> Sqrt; no
    normalization is ever materialized.
  * The 16-step affine flow scan has the closed form
      z = exp(S) * x + sum_j exp(sum_{k>j} sc_k) * t_j
    computed with one triangular-matrix matmul.
  * Output is a scalar; each core emits 48 partial sums, host combines.

Sharding: the N=1024 pixels are split across 8 cores (128 each); all weights
are replicated.  Device layout keeps features on partitions and pixels on the
free dimension, so no transposes are ever needed.  The FFN (5->2048->5) and
the rank-1 bias terms run in bf16 (fp32 accumulation); the main residual
path stays fp32.
"""

import numpy as np

B, H, W = 32, 32, 32
N = H * W
L, D, FF = 8, 5, 2048
NCORES = 8
NP = N // NCORES          # pixels per core
NCHUNK = FF // 128        # 16 ff chunks of 128
NQ = NCHUNK // 4          # 4 chunks batched per PSUM bank
EPS = 1e-5

_PROG = None              # cached compiled Bass program


def _build_program():
    import concourse.bacc as bacc
    import concourse.mybir as mybir
    import concourse.tile as tile

    f32 = mybir.dt.float32
    bf16 = mybir.dt.bfloat16
    AF = mybir.ActivationFunctionType
    ALU = mybir.AluOpType
    AX = mybir.AxisListType

    nc = bacc.Bacc(name="pixel_transformer")

    smalls_d = nc.dram_tensor("smalls", [5, 80], f32, kind="ExternalInput")
    brows_d = nc.dram_tensor("brows", [1, 81], bf16, kind="ExternalInput")
    hrows_d = nc.dram_tensor("hrows", [1, 48], f32, kind="ExternalInput")
    smallsb_d = nc.dram_tensor("smallsb", [5, 40], bf16, kind="ExternalInput")
    w1b_d = [
        nc.dram_tensor(f"w1b{l}", [5, FF], bf16, kind="ExternalInput")
        for l in range(L)
    ]
    b1b_d = [
        nc.dram_tensor(f"b1b{l}", [1, FF], bf16, kind="ExternalInput")
        for l in range(L)
    ]
    w2b_d = nc.dram_tensor("w2b", [128, L * NCHUNK * D], bf16, kind="ExternalInput")
    headw_d = nc.dram_tensor("headw", [16, 66], f32, kind="ExternalInput")
    tok_d = nc.dram_tensor("tok", [5, NP], f32, kind="ExternalInput")
    xsh_d = nc.dram_tensor("xsh", [B, NP], f32, kind="ExternalInput")
    out_d = nc.dram_tensor("out", [48, 1], f32, kind="ExternalOutput")

    with tile.TileContext(nc) as tc:
        with (
            tc.tile_pool(name="consts", bufs=1) as cp,
            tc.tile_pool(name="work", bufs=2) as wp,
            tc.tile_pool(name="fsb", bufs=3) as fp,
            tc.tile_pool(name="ps", bufs=2, space="PSUM") as pp,
        ):
            # --- input DMAs, spread across SP / Pool / ACT engines ---
            w1sb = []
            b1sb = []
            for l in range(L):
                t = cp.tile([5, FF], bf16, name=f"w1sb{l}")
                tb = cp.tile([1, FF], bf16, name=f"b1sb{l}")
                eng = nc.sync if l % 2 == 0 else nc.gpsimd
                eng.dma_start(out=t, in_=w1b_d[l][:, :])
                eng.dma_start(out=tb, in_=b1b_d[l][:, :])
                w1sb.append(t)
                b1sb.append(tb)
            smalls = cp.tile([5, 80], f32)
            nc.sync.dma_start(out=smalls, in_=smalls_d[:, :])
            brows = cp.tile([1, 81], bf16)
            nc.sync.dma_start(out=brows, in_=brows_d[:, :])
            toksb = cp.tile([5, NP], f32)
            nc.sync.dma_start(out=toksb, in_=tok_d[:, :])
            w2sb = cp.tile([128, L * NCHUNK * D], bf16)
            nc.scalar.dma_start(out=w2sb, in_=w2b_d[:, :])
            headsb = cp.tile([16, 66], f32)
            nc.scalar.dma_start(out=headsb, in_=headw_d[:, :])
            hrows = cp.tile([1, 48], f32)
            nc.scalar.dma_start(out=hrows, in_=hrows_d[:, :])
            smallsb = cp.tile([5, 40], bf16)
            nc.scalar.dma_start(out=smallsb, in_=smallsb_d[:, :])
            xsb = cp.tile([B, NP], f32)
            nc.scalar.dma_start(out=xsb, in_=xsh_d[:, :])

            # --- constants via memset ---
            vconst = cp.tile([5, 1], f32)       # 1/D rows for variance matmul
            nc.vector.memset(vconst, 1.0 / D)
            ones16 = cp.tile([16, 1], f32)      # column-sum over 16 partitions
            nc.vector.memset(ones16, 1.0)
            ones_b16 = cp.tile([1, 16], f32)    # broadcast 1 -> 16 partitions
            nc.vector.memset(ones_b16, 1.0)
            ones_b32 = cp.tile([1, B], f32)     # broadcast 1 -> 32 partitions
            nc.vector.memset(ones_b32, 1.0)
            std0b = cp.tile([1, NP], bf16)      # std before layer 0 == 1
            nc.vector.memset(std0b, 1.0)
            psv0b = cp.tile([1, NP], bf16)      # var+eps proxy before layer 0
            nc.vector.memset(psv0b, 1.0)

            def ln_v2(ps_y, psvb_prev, idx, fp32_std=False,
                      want_psvb=True, want_u=True):
                """LN in the scaled representation: unscaled centered PSUM
                ps_y -> (u [5,NP] f32, std bf16 (or f32), psv bf16)."""
                sq = wp.tile([5, NP], f32, tag="sq", name=f"sq{idx}")
                nc.scalar.activation(out=sq, in_=ps_y, func=AF.Square)
                psv = pp.tile([1, NP], f32, tag="psmall", name=f"psv{idx}")
                nc.tensor.matmul(
                    psv, brows[:, 80:81], psvb_prev, start=True, stop=False
                )
                nc.tensor.matmul(psv, vconst, sq, start=False, stop=True)
                u = None
                if want_u:
                    u = wp.tile([5, NP], f32, tag="u", bufs=3, name=f"u{idx}")
                    nc.vector.tensor_copy(out=u, in_=ps_y)
                stdt = f32 if fp32_std else bf16
                stdb = wp.tile([1, NP], stdt, tag="stdb", bufs=3,
                               name=f"stdb{idx}")
                nc.scalar.activation(out=stdb, in_=psv, func=AF.Sqrt)
                psvb = None
                if want_psvb:
                    psvb = wp.tile([1, NP], bf16, tag="psvb", bufs=3,
                                   name=f"psvb{idx}")
                    nc.vector.tensor_copy(out=psvb, in_=psv)
                return u, stdb, psvb

            u_prev, stdb_prev, psvb_prev = toksb, std0b, psv0b
            for l in range(L):
                # attention (folded) + residual + LN1 centering
                psy = pp.tile([D, NP], f32, tag="psmall", name=f"psy{l}")
                nc.tensor.matmul(
                    psy, smalls[:, 5 * l:5 * l + 5], u_prev,
                    start=True, stop=False,
                )
                nc.tensor.matmul(
                    psy, brows[:, 5 * l:5 * l + 5], stdb_prev,
                    start=False, stop=True,
                )
                u1b = wp.tile([5, NP], bf16, tag="ub", name=f"ub{l}")
                nc.vector.tensor_copy(out=u1b, in_=psy)
                _, std1b, psvb1 = ln_v2(psy, psvb_prev, f"a{l}", want_u=False)

                # FFN in bf16; psum_y2 accumulates the centered layer output
                psy2 = pp.tile([D, NP], f32, tag="pacc", bufs=2, name=f"psy2{l}")
                nc.tensor.matmul(
                    psy2, smallsb[:, 5 * l:5 * l + 5], u1b,
                    start=True, stop=False,
                )
                # all mm1 mains first (only need u1b, ready early); the
                # rank-1 bias matmuls wait on sqrt -> keep them behind so
                # they don't block the mains in PE program order
                psfs = []
                for q in range(NQ):
                    psf = pp.tile([128, 512], f32, tag="pf", bufs=4,
                                  name=f"psf{l}_{q}")
                    psfs.append(psf)
                    for c4 in range(4):
                        c = 4 * q + c4
                        nc.tensor.matmul(
                            psf[:, 128 * c4:128 * (c4 + 1)],
                            w1sb[l][:, 128 * c:128 * (c + 1)],
                            u1b, start=(c4 == 0), stop=False,
                        )
                for q in range(NQ):
                    for c4 in range(4):
                        c = 4 * q + c4
                        nc.tensor.matmul(
                            psfs[q][:, 128 * c4:128 * (c4 + 1)],
                            b1sb[l][:, 128 * c:128 * (c + 1)],
                            std1b, start=False, stop=(c4 == 3),
                        )
                nc.tensor.matmul(
                    psy2, brows[:, 40 + 5 * l:45 + 5 * l], std1b,
                    start=False, stop=False,
                )
                for q in range(NQ):
                    fq = fp.tile([128, 512], bf16, tag="f", name=f"f{l}_{q}")
                    if q % 2 == 0:
                        nc.scalar.activation(out=fq, in_=psfs[q], func=AF.Relu)
                    else:
                        nc.vector.tensor_scalar(
                            out=fq, in0=psfs[q], scalar1=0.0, scalar2=None,
                            op0=ALU.max,
                        )
                    for c4 in range(4):
                        c = 4 * q + c4
                        nc.tensor.matmul(
                            psy2,
                            w2sb[:, (NCHUNK * l + c) * D:(NCHUNK * l + c + 1) * D],
                            fq[:, 128 * c4:128 * (c4 + 1)],
                            start=False,
                            stop=(c == NCHUNK - 1),
                        )
                u_prev, stdb_prev, psvb_prev = ln_v2(
                    psy2, psvb1, f"b{l}", fp32_std=(l == L - 1),
                    want_psvb=(l < L - 1),
                )
            std8 = stdb_prev

            # ---- head ----
            # hid_u = relu(f0w1'@u8 + f0b1' x std8)
            psh = pp.tile([16, NP], f32, tag="psmall")
            nc.tensor.matmul(psh, headsb[0:5, 0:16], u_prev, start=True, stop=False)
            nc.tensor.matmul(psh, hrows[:, 0:16], std8, start=False, stop=True)
            hid = wp.tile([16, NP], f32, tag="sq")
            nc.scalar.activation(out=hid, in_=psh, func=AF.Relu)

            # st halves, still scaled by 1/std8
            psss = pp.tile([16, NP], f32, tag="psmall")
            nc.tensor.matmul(psss, headsb[:, 16:32], hid, start=True, stop=False)
            nc.tensor.matmul(psss, hrows[:, 16:32], std8, start=False, stop=True)
            psst = pp.tile([16, NP], f32, tag="pf", bufs=4)
            nc.tensor.matmul(psst, headsb[:, 32:48], hid, start=True, stop=False)
            nc.tensor.matmul(psst, hrows[:, 32:48], std8, start=False, stop=True)

            # materialize absolute st: r8 = 1/std8 broadcast to 16 partitions
            rec8 = wp.tile([1, NP], f32, tag="rec8")
            nc.vector.reciprocal(out=rec8, in_=std8)
            psbc = pp.tile([16, NP], f32, tag="pacc", bufs=2)
            nc.tensor.matmul(psbc, ones_b16, rec8, start=True, stop=True)
            r8bc = wp.tile([16, NP], f32, tag="r8bc")
            nc.vector.tensor_copy(out=r8bc, in_=psbc)
            s_sb = wp.tile([16, NP], f32, tag="u", bufs=3)
            nc.vector.tensor_mul(out=s_sb, in0=psss, in1=r8bc)
            t_sb = wp.tile([16, NP], f32, tag="ub")
            nc.vector.tensor_mul(out=t_sb, in0=psst, in1=r8bc)

            ssum = wp.tile([16, 1], f32, tag="ssum")
            nc.vector.reduce_sum(out=ssum, in_=s_sb, axis=AX.X)

            sf = wp.tile([16, 1], f32, tag="sf")
            nc.scalar.activation(out=sf, in_=headsb[0:16, 48:49], func=AF.Exp)
            rsf = wp.tile([16, 1], f32, tag="rsf")
            nc.vector.reciprocal(out=rsf, in_=sf)
            sc1 = wp.tile([16, NP], f32, tag="sc1")
            nc.vector.tensor_scalar(
                out=sc1, in0=s_sb, scalar1=rsf, scalar2=None, op0=ALU.mult
            )
            th = wp.tile([16, NP], f32, tag="th")
            nc.scalar.activation(out=th, in_=sc1, func=AF.Tanh)
            sc = wp.tile([16, NP], f32, tag="sc")
            nc.vector.tensor_scalar(
                out=sc, in0=th, scalar1=sf, scalar2=None, op0=ALU.mult
            )

            # flow closed form
            psD = pp.tile([16, NP], f32, tag="psmall")
            nc.tensor.matmul(psD, headsb[0:16, 49:65], sc, start=True, stop=True)
            psS = pp.tile([1, NP], f32, tag="pf", bufs=4)
            nc.tensor.matmul(psS, ones16, sc, start=True, stop=True)
            wexp = wp.tile([16, NP], f32, tag="sc1")
            nc.scalar.activation(out=wexp, in_=psD, func=AF.Exp)
            eS = wp.tile([1, NP], f32, tag="eS")
            nc.scalar.activation(out=eS, in_=psS, func=AF.Exp)
            wt = wp.tile([16, NP], f32, tag="th")
            nc.vector.tensor_mul(out=wt, in0=wexp, in1=t_sb)
            psc1 = pp.tile([1, NP], f32, tag="psmall")
            nc.tensor.matmul(psc1, ones16, wt, start=True, stop=True)
            c1sb = wp.tile([1, NP], f32, tag="c1")
            nc.vector.tensor_copy(out=c1sb, in_=psc1)

            pseb = pp.tile([B, NP], f32, tag="pacc", bufs=2)
            nc.tensor.matmul(pseb, ones_b32, eS, start=True, stop=True)
            pscb = pp.tile([B, NP], f32, tag="psmall")
            nc.tensor.matmul(pscb, ones_b32, c1sb, start=True, stop=True)

            zt = wp.tile([B, NP], f32, tag="zt")
            nc.vector.tensor_mul(out=zt, in0=xsb, in1=pseb)
            z = wp.tile([B, NP], f32, tag="z")
            nc.vector.tensor_add(out=z, in0=zt, in1=pscb)
            zsq = wp.tile([B, NP], f32, tag="zt2")
            nc.scalar.activation(out=zsq, in_=z, func=AF.Square)
            zrow = wp.tile([B, 1], f32, tag="zrow")
            nc.vector.reduce_sum(out=zrow, in_=zsq, axis=AX.X)

            nc.sync.dma_start(out=out_d[0:16, :], in_=ssum)
            nc.sync.dma_start(out=out_d[16:48, :], in_=zrow)

    nc.finalize()
    return nc


def _fold_inputs(inp):
    """Host-side weight folding (float64 for precision, cast at the end)."""
    import ml_dtypes

    C = np.eye(D) - np.ones((D, D)) / D
    g = lambda k: np.asarray(inp[k], dtype=np.float64)
    wqkv, bqkv, wo, bo = g("wqkv"), g("bqkv"), g("wo"), g("bo")
    w1, b1, w2, b2 = g("w1"), g("b1"), g("w2"), g("b2")
    ln1w, ln1b, ln2w, ln2b = g("ln1w"), g("ln1b"), g("ln2w"), g("ln2b")

    smalls = np.zeros((5, 80), np.float64)
    brows = np.zeros((1, 81), np.float64)
    w1a = []
    b1b = np.zeros((L, 1, FF), np.float64)
    w2b = np.zeros((128, L * NCHUNK * D), np.float64)
    for l in range(L):
        Dl = np.diag(ln2w[l - 1]) if l > 0 else np.eye(D)
        el = ln2b[l - 1] if l > 0 else np.zeros(D)
        wv = wqkv[l][2 * D:3 * D, :]
        bv = bqkv[l][2 * D:3 * D]
        A0 = np.eye(D) + wo[l] @ wv
        c_attn = wo[l] @ bv + bo[l]
        smalls[:, 5 * l:5 * l + 5] = (C @ A0 @ Dl).T
        brows[0, 5 * l:5 * l + 5] = C @ (A0 @ el + c_attn)
        smalls[:, 40 + 5 * l:45 + 5 * l] = (C @ np.diag(ln1w[l])).T
        brows[0, 40 + 5 * l:45 + 5 * l] = C @ (ln1b[l] + b2[l])
        w1a.append((w1[l] * ln1w[l][None, :]).T)       # [5, FF]
        b1b[l, 0, :] = b1[l] + w1[l] @ ln1b[l]
        w2full = (C @ w2[l]).T                         # [2048, 5]
        for c in range(NCHUNK):
            w2b[:, (NCHUNK * l + c) * D:(NCHUNK * l + c + 1) * D] = (
                w2full[128 * c:128 * (c + 1), :]
            )
    brows[0, 80] = EPS

    f0w1, f0b1 = g("f0w1"), g("f0b1")
    f0w2, f0b2 = g("f0w2"), g("f0b2")
    D8 = np.diag(ln2w[L - 1])
    e8 = ln2b[L - 1]
    headw = np.zeros((16, 66), np.float64)
    headw[0:5, 0:16] = (f0w1 @ D8).T
    headw[0:16, 16:32] = f0w2.T[:, 0:16]
    headw[0:16, 32:48] = f0w2.T[:, 16:32]
    headw[0:16, 48] = float(np.asarray(inp["sfac"])[0])
    for j in range(16):
        headw[j + 1:16, 49 + j] = 1.0                 # sum_{k>j}
    hrows = np.zeros((1, 48), np.float64)
    hrows[0, 0:16] = f0b1 + f0w1 @ e8
    hrows[0, 16:32] = f0b2[0:16]
    hrows[0, 32:48] = f0b2[16:32]

    # positional tokens, exactly as the reference builds them (fp32 ops)
    xs = (np.arange(W, dtype=np.float32) / np.float32(1e4)).astype(np.float32)
    ys = (np.arange(H, dtype=np.float32) / np.float32(1e4)).astype(np.float32)
    sinx = np.broadcast_to(np.sin(xs)[None, :], (H, W)).reshape(N)
    cosx = np.broadcast_to(np.cos(xs)[None, :], (H, W)).reshape(N)
    siny = np.broadcast_to(np.sin(ys)[:, None], (H, W)).reshape(N)
    cosy = np.broadcast_to(np.cos(ys)[:, None], (H, W)).reshape(N)
    tok = np.stack(
        [-np.ones(N, np.float32), sinx, cosx, siny, cosy], axis=0
    )                                                  # [5, N]
    xflat = np.asarray(inp["x"], dtype=np.float32)[:, 0].reshape(B, N)

    out = {
        "smalls": smalls.astype(np.float32),
        "smallsb": smalls[:, 40:80].astype(ml_dtypes.bfloat16),
        "brows": brows.astype(ml_dtypes.bfloat16),
        "hrows": hrows.astype(np.float32),
        "w2b": w2b.astype(ml_dtypes.bfloat16),
        "headw": headw.astype(np.float32),
        "tok": tok.astype(np.float32),
        "xsh": xflat,
    }
    for l in range(L):
        out[f"w1b{l}"] = w1a[l].astype(ml_dtypes.bfloat16)
        out[f"b1b{l}"] = b1b[l].astype(ml_dtypes.bfloat16)
    return out


def get_program():
    global _PROG
    if _PROG is None:
        _PROG = _build_program()
    return _PROG


def make_in_maps(inputs):
    arrs = _fold_inputs(inputs)
    shared_keys = (
        ["smalls", "smallsb", "brows", "hrows", "w2b", "headw"]
        + [f"w1b{l}" for l in range(L)]
        + [f"b1b{l}" for l in range(L)]
    )
    shared = {k: arrs[k] for k in shared_keys}
    in_maps = []
    for core in range(NCORES):
        sl = slice(core * NP, (core + 1) * NP)
        m = dict(shared)
        m["tok"] = np.ascontiguousarray(arrs["tok"][:, sl])
        m["xsh"] = np.ascontiguousarray(arrs["xsh"][:, sl])
        in_maps.append(m)
    return in_maps


def combine_outputs(outs):
    """outs: list of per-core [48, 1] arrays -> scalar float32."""
    s_tot = 0.0
    q_tot = 0.0
    for o in outs:
        o = np.asarray(o, dtype=np.float64).reshape(48)
        s_tot += o[0:16].sum()
        q_tot += o[16:48].sum()
    sldj = B * s_tot - 0.5 * q_tot - B * N * 0.5 * np.log(2.0 * np.pi)
    return np.array(-sldj, dtype=np.float32)


def kernel(**inputs):
    from concourse.bass_utils import run_bass_kernel_spmd

    nc = get_program()
    in_maps = make_in_maps(inputs)
    res = run_bass_kernel_spmd(nc, in_maps, core_ids=list(range(NCORES)))
    return combine_outputs([r["out"] for r in res.results])


# revision 16
# speedup vs baseline: 1.2784x; 1.1668x over previous
"""Trainium2 Bass kernel for nn_PixelTransformer.

Math notes (derived from the reference semantics, valid for ANY input values):
  * The transformer hidden state is built purely from positional encodings
    (x never enters it), broadcast over the batch.  The attention mixes only
    across the batch axis (head_dim=1), so with identical tokens per batch the
    softmax is uniform and the attention output equals v exactly.  Attention +
    residual therefore folds into a per-layer 5x5 linear map.
  * LayerNorm centering is a linear projection C = I - J/5, foldable into the
    preceding matmuls; LN affine params fold into the following matmuls.
  * LayerNorm is invariant to per-pixel positive scaling, and ReLU commutes
    with it.  The kernel therefore keeps the state in an UNSCALED
    representation u with g_true = (1/std) * u, tracking std per pixel.
    Biases become rank-1 accumulating matmuls (bias_row x std), and the eps
    of 1/sqrt(var+eps) becomes a rank-1 term (eps x std_prev^2) in the
    variance matmul.  Each LN is then just Square -> matmul -> Sqrt; no
    normalization is ever materialized.
  * The 16-step affine flow scan has the closed form
      z = exp(S) * x + sum_j exp(sum_{k>j} sc_k) * t_j
    computed with one triangular-matrix matmul.
  * Output is a scalar; each core emits 48 partial sums, host combines.

Sharding: the N=1024 pixels are split across 8 cores (128 each); all weights
are replicated.  Device layout keeps features on partitions and pixels on the
free dimension, so no transposes are ever needed.  The FFN (5->2048->5) and
the rank-1 bias terms run in bf16 (fp32 accumulation); the main residual
path stays fp32.
"""

import numpy as np

B, H, W = 32, 32, 32
N = H * W
L, D, FF = 8, 5, 2048
NCORES = 8
NP = N // NCORES          # pixels per core
NCHUNK = FF // 128        # 16 ff chunks of 128
NQ = NCHUNK // 4          # 4 chunks batched per PSUM bank
EPS = 1e-5

_PROG = None              # cached compiled Bass program


def _build_program():
    import concourse.bacc as bacc
    import concourse.mybir as mybir
    import concourse.tile as tile

    f32 = mybir.dt.float32
    bf16 = mybir.dt.bfloat16
    AF = mybir.ActivationFunctionType
    ALU = mybir.AluOpType
    AX = mybir.AxisListType

    nc = bacc.Bacc(name="pixel_transformer")

    smalls_d = nc.dram_tensor("smalls", [5, 80], f32, kind="ExternalInput")
    brows_d = nc.dram_tensor("brows", [1, 81], bf16, kind="ExternalInput")
    hrows_d = nc.dram_tensor("hrows", [1, 48], f32, kind="ExternalInput")
    smallsb_d = nc.dram_tensor("smallsb", [5, 40], bf16, kind="ExternalInput")
    w1b_d = [
        nc.dram_tensor(f"w1b{l}", [5, FF], bf16, kind="ExternalInput")
        for l in range(L)
    ]
    b1b_d = [
        nc.dram_tensor(f"b1b{l}", [1, FF], bf16, kind="ExternalInput")
        for l in range(L)
    ]
    w2b_d = nc.dram_tensor("w2b", [128, L * NCHUNK * D], bf16, kind="ExternalInput")
    headw_d = nc.dram_tensor("headw", [16, 66], f32, kind="ExternalInput")
    tok_d = nc.dram_tensor("tok", [5, NP], f32, kind="ExternalInput")
    xsh_d = nc.dram_tensor("xsh", [B, NP], f32, kind="ExternalInput")
    out_d = nc.dram_tensor("out", [48, 1], f32, kind="ExternalOutput")

    with tile.TileContext(nc) as tc:
        with (
            tc.tile_pool(name="consts", bufs=1) as cp,
            tc.tile_pool(name="work", bufs=2) as wp,
            tc.tile_pool(name="fsb", bufs=3) as fp,
            tc.tile_pool(name="ps", bufs=2, space="PSUM") as pp,
        ):
            # --- input DMAs ---
            # SP: layer-0 weights + small critical tensors first, then the
            # per-layer w1 / b1-low-half stream.  Pool: w2 + all b1 high
            # halves (early).  ACT: b1-low of layer 0 only; head tensors are
            # DMA'd mid-program.
            HFF = FF // 2
            w1sb = []
            b1lo = []
            b1hi = []
            for l in range(L):
                w1sb.append(cp.tile([5, FF], bf16, name=f"w1sb{l}"))
                b1lo.append(cp.tile([1, HFF], bf16, name=f"b1lo{l}"))
                b1hi.append(cp.tile([1, HFF], bf16, name=f"b1hi{l}"))
            nc.sync.dma_start(out=w1sb[0], in_=w1b_d[0][:, :])
            toksb = cp.tile([5, NP], f32)
            nc.sync.dma_start(out=toksb, in_=tok_d[:, :])
            smalls = cp.tile([5, 80], f32)
            nc.sync.dma_start(out=smalls, in_=smalls_d[:, :])
            brows = cp.tile([1, 81], bf16)
            nc.sync.dma_start(out=brows, in_=brows_d[:, :])
            for l in range(1, L):
                nc.sync.dma_start(out=b1lo[l], in_=b1b_d[l][:, 0:HFF])
                nc.sync.dma_start(out=w1sb[l], in_=w1b_d[l][:, :])
            w2sb = cp.tile([128, L * NCHUNK * D], bf16)
            nc.gpsimd.dma_start(out=w2sb, in_=w2b_d[:, :])
            smallsb = cp.tile([5, 40], bf16)
            nc.gpsimd.dma_start(out=smallsb, in_=smallsb_d[:, :])
            for l in range(L):
                nc.gpsimd.dma_start(out=b1hi[l], in_=b1b_d[l][:, HFF:FF])
            nc.scalar.dma_start(out=b1lo[0], in_=b1b_d[0][:, 0:HFF])
            # head tensors declared here, DMA'd later (mid-program)
            headsb = cp.tile([16, 66], f32)
            hrows = cp.tile([1, 48], f32)
            xsb = cp.tile([B, NP], f32)

            # --- constants via memset ---
            vconst = cp.tile([5, 1], f32)       # 1/D rows for variance matmul
            nc.vector.memset(vconst, 1.0 / D)
            ones16 = cp.tile([16, 1], f32)      # column-sum over 16 partitions
            nc.vector.memset(ones16, 1.0)
            ones_b16 = cp.tile([1, 16], f32)    # broadcast 1 -> 16 partitions
            nc.vector.memset(ones_b16, 1.0)
            ones_b32 = cp.tile([1, B], f32)     # broadcast 1 -> 32 partitions
            nc.vector.memset(ones_b32, 1.0)
            std0b = cp.tile([1, NP], bf16)      # std before layer 0 == 1
            nc.vector.memset(std0b, 1.0)
            psv0b = cp.tile([1, NP], bf16)      # var+eps proxy before layer 0
            nc.vector.memset(psv0b, 1.0)

            def ln_v2(ps_y, psvb_prev, idx, fp32_std=False,
                      want_psvb=True, want_u=True):
                """LN in the scaled representation: unscaled centered PSUM
                ps_y -> (u [5,NP] f32, std bf16 (or f32), psv bf16)."""
                sq = wp.tile([5, NP], f32, tag="sq", name=f"sq{idx}")
                nc.scalar.activation(out=sq, in_=ps_y, func=AF.Square)
                psv = pp.tile([1, NP], f32, tag="psmall", name=f"psv{idx}")
                nc.tensor.matmul(
                    psv, brows[:, 80:81], psvb_prev, start=True, stop=False
                )
                nc.tensor.matmul(psv, vconst, sq, start=False, stop=True)
                u = None
                if want_u:
                    u = wp.tile([5, NP], f32, tag="u", bufs=3, name=f"u{idx}")
                    nc.vector.tensor_copy(out=u, in_=ps_y)
                stdt = f32 if fp32_std else bf16
                stdb = wp.tile([1, NP], stdt, tag="stdb", bufs=3,
                               name=f"stdb{idx}")
                nc.scalar.activation(out=stdb, in_=psv, func=AF.Sqrt)
                psvb = None
                if want_psvb:
                    psvb = wp.tile([1, NP], bf16, tag="psvb", bufs=3,
                                   name=f"psvb{idx}")
                    nc.vector.tensor_copy(out=psvb, in_=psv)
                return u, stdb, psvb

            u_prev, stdb_prev, psvb_prev = toksb, std0b, psv0b
            for l in range(L):
                # attention (folded) + residual + LN1 centering
                psy = pp.tile([D, NP], f32, tag="psmall", name=f"psy{l}")
                nc.tensor.matmul(
                    psy, smalls[:, 5 * l:5 * l + 5], u_prev,
                    start=True, stop=False,
                )
                nc.tensor.matmul(
                    psy, brows[:, 5 * l:5 * l + 5], stdb_prev,
                    start=False, stop=True,
                )
                u1b = wp.tile([5, NP], bf16, tag="ub", name=f"ub{l}")
                nc.vector.tensor_copy(out=u1b, in_=psy)
                _, std1b, psvb1 = ln_v2(psy, psvb_prev, f"a{l}", want_u=False)

                # FFN in bf16; psum_y2 accumulates the centered layer output
                psy2 = pp.tile([D, NP], f32, tag="pacc", bufs=2, name=f"psy2{l}")
                nc.tensor.matmul(
                    psy2, smallsb[:, 5 * l:5 * l + 5], u1b,
                    start=True, stop=False,
                )
                # all mm1 mains first (only need u1b, ready early); the
                # rank-1 bias matmuls wait on sqrt -> keep them behind so
                # they don't block the mains in PE program order
                psfs = []
                for q in range(NQ):
                    psf = pp.tile([128, 512], f32, tag="pf", bufs=4,
                                  name=f"psf{l}_{q}")
                    psfs.append(psf)
                    for c4 in range(4):
                        c = 4 * q + c4
                        nc.tensor.matmul(
                            psf[:, 128 * c4:128 * (c4 + 1)],
                            w1sb[l][:, 128 * c:128 * (c + 1)],
                            u1b, start=(c4 == 0), stop=False,
                        )
                for q in range(NQ):
                    for c4 in range(4):
                        c = 4 * q + c4
                        bsrc = b1lo[l] if c < 8 else b1hi[l]
                        cc = c % 8
                        nc.tensor.matmul(
                            psfs[q][:, 128 * c4:128 * (c4 + 1)],
                            bsrc[:, 128 * cc:128 * (cc + 1)],
                            std1b, start=False, stop=(c4 == 3),
                        )
                nc.tensor.matmul(
                    psy2, brows[:, 40 + 5 * l:45 + 5 * l], std1b,
                    start=False, stop=False,
                )
                for q in range(NQ):
                    fq = fp.tile([128, 512], bf16, tag="f", name=f"f{l}_{q}")
                    if q % 2 == 0:
                        nc.scalar.activation(out=fq, in_=psfs[q], func=AF.Relu)
                    else:
                        nc.vector.tensor_scalar(
                            out=fq, in0=psfs[q], scalar1=0.0, scalar2=None,
                            op0=ALU.max,
                        )
                    for c4 in range(4):
                        c = 4 * q + c4
                        nc.tensor.matmul(
                            psy2,
                            w2sb[:, (NCHUNK * l + c) * D:(NCHUNK * l + c + 1) * D],
                            fq[:, 128 * c4:128 * (c4 + 1)],
                            start=False,
                            stop=(c == NCHUNK - 1),
                        )
                u_prev, stdb_prev, psvb_prev = ln_v2(
                    psy2, psvb1, f"b{l}", fp32_std=(l == L - 1),
                    want_psvb=(l < L - 1),
                )
                if l == 4:
                    nc.sync.dma_start(out=headsb, in_=headw_d[:, :])
                    nc.sync.dma_start(out=hrows, in_=hrows_d[:, :])
                    nc.sync.dma_start(out=xsb, in_=xsh_d[:, :])
            std8 = stdb_prev

            # ---- head ----
            # hid_u = relu(f0w1'@u8 + f0b1' x std8)
            psh = pp.tile([16, NP], f32, tag="psmall")
            nc.tensor.matmul(psh, headsb[0:5, 0:16], u_prev, start=True, stop=False)
            nc.tensor.matmul(psh, hrows[:, 0:16], std8, start=False, stop=True)
            hid = wp.tile([16, NP], f32, tag="sq")
            nc.scalar.activation(out=hid, in_=psh, func=AF.Relu)

            # st halves, still scaled by 1/std8
            psss = pp.tile([16, NP], f32, tag="psmall")
            nc.tensor.matmul(psss, headsb[:, 16:32], hid, start=True, stop=False)
            nc.tensor.matmul(psss, hrows[:, 16:32], std8, start=False, stop=True)
            psst = pp.tile([16, NP], f32, tag="pf", bufs=4)
            nc.tensor.matmul(psst, headsb[:, 32:48], hid, start=True, stop=False)
            nc.tensor.matmul(psst, hrows[:, 32:48], std8, start=False, stop=True)

            # materialize absolute st: r8 = 1/std8 broadcast to 16 partitions
            rec8 = wp.tile([1, NP], f32, tag="rec8")
            nc.vector.reciprocal(out=rec8, in_=std8)
            psbc = pp.tile([16, NP], f32, tag="pacc", bufs=2)
            nc.tensor.matmul(psbc, ones_b16, rec8, start=True, stop=True)
            r8bc = wp.tile([16, NP], f32, tag="r8bc")
            nc.vector.tensor_copy(out=r8bc, in_=psbc)
            s_sb = wp.tile([16, NP], f32, tag="u", bufs=3)
            nc.vector.tensor_mul(out=s_sb, in0=psss, in1=r8bc)
            t_sb = wp.tile([16, NP], f32, tag="ub")
            nc.vector.tensor_mul(out=t_sb, in0=psst, in1=r8bc)

            ssum = wp.tile([16, 1], f32, tag="ssum")
            nc.vector.reduce_sum(out=ssum, in_=s_sb, axis=AX.X)

            sf = wp.tile([16, 1], f32, tag="sf")
            nc.scalar.activation(out=sf, in_=headsb[0:16, 48:49], func=AF.Exp)
            rsf = wp.tile([16, 1], f32, tag="rsf")
            nc.vector.reciprocal(out=rsf, in_=sf)
            sc1 = wp.tile([16, NP], f32, tag="sc1")
            nc.vector.tensor_scalar(
                out=sc1, in0=s_sb, scalar1=rsf, scalar2=None, op0=ALU.mult
            )
            th = wp.tile([16, NP], f32, tag="th")
            nc.scalar.activation(out=th, in_=sc1, func=AF.Tanh)
            sc = wp.tile([16, NP], f32, tag="sc")
            nc.vector.tensor_scalar(
                out=sc, in0=th, scalar1=sf, scalar2=None, op0=ALU.mult
            )

            # flow closed form
            psD = pp.tile([16, NP], f32, tag="psmall")
            nc.tensor.matmul(psD, headsb[0:16, 49:65], sc, start=True, stop=True)
            psS = pp.tile([1, NP], f32, tag="pf", bufs=4)
            nc.tensor.matmul(psS, ones16, sc, start=True, stop=True)
            wexp = wp.tile([16, NP], f32, tag="sc1")
            nc.scalar.activation(out=wexp, in_=psD, func=AF.Exp)
            eS = wp.tile([1, NP], f32, tag="eS")
            nc.scalar.activation(out=eS, in_=psS, func=AF.Exp)
            wt = wp.tile([16, NP], f32, tag="th")
            nc.vector.tensor_mul(out=wt, in0=wexp, in1=t_sb)
            psc1 = pp.tile([1, NP], f32, tag="psmall")
            nc.tensor.matmul(psc1, ones16, wt, start=True, stop=True)
            c1sb = wp.tile([1, NP], f32, tag="c1")
            nc.vector.tensor_copy(out=c1sb, in_=psc1)

            pseb = pp.tile([B, NP], f32, tag="pacc", bufs=2)
            nc.tensor.matmul(pseb, ones_b32, eS, start=True, stop=True)
            pscb = pp.tile([B, NP], f32, tag="psmall")
            nc.tensor.matmul(pscb, ones_b32, c1sb, start=True, stop=True)

            zt = wp.tile([B, NP], f32, tag="zt")
            nc.vector.tensor_mul(out=zt, in0=xsb, in1=pseb)
            z = wp.tile([B, NP], f32, tag="z")
            nc.vector.tensor_add(out=z, in0=zt, in1=pscb)
            zsq = wp.tile([B, NP], f32, tag="zt2")
            nc.scalar.activation(out=zsq, in_=z, func=AF.Square)
            zrow = wp.tile([B, 1], f32, tag="zrow")
            nc.vector.reduce_sum(out=zrow, in_=zsq, axis=AX.X)

            nc.sync.dma_start(out=out_d[0:16, :], in_=ssum)
            nc.sync.dma_start(out=out_d[16:48, :], in_=zrow)

    nc.finalize()
    return nc


def _fold_inputs(inp):
    """Host-side weight folding (float64 for precision, cast at the end)."""
    import ml_dtypes

    C = np.eye(D) - np.ones((D, D)) / D
    g = lambda k: np.asarray(inp[k], dtype=np.float64)
    wqkv, bqkv, wo, bo = g("wqkv"), g("bqkv"), g("wo"), g("bo")
    w1, b1, w2, b2 = g("w1"), g("b1"), g("w2"), g("b2")
    ln1w, ln1b, ln2w, ln2b = g("ln1w"), g("ln1b"), g("ln2w"), g("ln2b")

    smalls = np.zeros((5, 80), np.float64)
    brows = np.zeros((1, 81), np.float64)
    w1a = []
    b1b = np.zeros((L, 1, FF), np.float64)
    w2b = np.zeros((128, L * NCHUNK * D), np.float64)
    for l in range(L):
        Dl = np.diag(ln2w[l - 1]) if l > 0 else np.eye(D)
        el = ln2b[l - 1] if l > 0 else np.zeros(D)
        wv = wqkv[l][2 * D:3 * D, :]
        bv = bqkv[l][2 * D:3 * D]
        A0 = np.eye(D) + wo[l] @ wv
        c_attn = wo[l] @ bv + bo[l]
        smalls[:, 5 * l:5 * l + 5] = (C @ A0 @ Dl).T
        brows[0, 5 * l:5 * l + 5] = C @ (A0 @ el + c_attn)
        smalls[:, 40 + 5 * l:45 + 5 * l] = (C @ np.diag(ln1w[l])).T
        brows[0, 40 + 5 * l:45 + 5 * l] = C @ (ln1b[l] + b2[l])
        w1a.append((w1[l] * ln1w[l][None, :]).T)       # [5, FF]
        b1b[l, 0, :] = b1[l] + w1[l] @ ln1b[l]
        w2full = (C @ w2[l]).T                         # [2048, 5]
        for c in range(NCHUNK):
            w2b[:, (NCHUNK * l + c) * D:(NCHUNK * l + c + 1) * D] = (
                w2full[128 * c:128 * (c + 1), :]
            )
    brows[0, 80] = EPS

    f0w1, f0b1 = g("f0w1"), g("f0b1")
    f0w2, f0b2 = g("f0w2"), g("f0b2")
    D8 = np.diag(ln2w[L - 1])
    e8 = ln2b[L - 1]
    headw = np.zeros((16, 66), np.float64)
    headw[0:5, 0:16] = (f0w1 @ D8).T
    headw[0:16, 16:32] = f0w2.T[:, 0:16]
    headw[0:16, 32:48] = f0w2.T[:, 16:32]
    headw[0:16, 48] = float(np.asarray(inp["sfac"])[0])
    for j in range(16):
        headw[j + 1:16, 49 + j] = 1.0                 # sum_{k>j}
    hrows = np.zeros((1, 48), np.float64)
    hrows[0, 0:16] = f0b1 + f0w1 @ e8
    hrows[0, 16:32] = f0b2[0:16]
    hrows[0, 32:48] = f0b2[16:32]

    # positional tokens, exactly as the reference builds them (fp32 ops)
    xs = (np.arange(W, dtype=np.float32) / np.float32(1e4)).astype(np.float32)
    ys = (np.arange(H, dtype=np.float32) / np.float32(1e4)).astype(np.float32)
    sinx = np.broadcast_to(np.sin(xs)[None, :], (H, W)).reshape(N)
    cosx = np.broadcast_to(np.cos(xs)[None, :], (H, W)).reshape(N)
    siny = np.broadcast_to(np.sin(ys)[:, None], (H, W)).reshape(N)
    cosy = np.broadcast_to(np.cos(ys)[:, None], (H, W)).reshape(N)
    tok = np.stack(
        [-np.ones(N, np.float32), sinx, cosx, siny, cosy], axis=0
    )                                                  # [5, N]
    xflat = np.asarray(inp["x"], dtype=np.float32)[:, 0].reshape(B, N)

    out = {
        "smalls": smalls.astype(np.float32),
        "smallsb": smalls[:, 40:80].astype(ml_dtypes.bfloat16),
        "brows": brows.astype(ml_dtypes.bfloat16),
        "hrows": hrows.astype(np.float32),
        "w2b": w2b.astype(ml_dtypes.bfloat16),
        "headw": headw.astype(np.float32),
        "tok": tok.astype(np.float32),
        "xsh": xflat,
    }
    for l in range(L):
        out[f"w1b{l}"] = w1a[l].astype(ml_dtypes.bfloat16)
        out[f"b1b{l}"] = b1b[l].astype(ml_dtypes.bfloat16)
    return out


def get_program():
    global _PROG
    if _PROG is None:
        _PROG = _build_program()
    return _PROG


def make_in_maps(inputs):
    arrs = _fold_inputs(inputs)
    shared_keys = (
        ["smalls", "smallsb", "brows", "hrows", "w2b", "headw"]
        + [f"w1b{l}" for l in range(L)]
        + [f"b1b{l}" for l in range(L)]
    )
    shared = {k: arrs[k] for k in shared_keys}
    in_maps = []
    for core in range(NCORES):
        sl = slice(core * NP, (core + 1) * NP)
        m = dict(shared)
        m["tok"] = np.ascontiguousarray(arrs["tok"][:, sl])
        m["xsh"] = np.ascontiguousarray(arrs["xsh"][:, sl])
        in_maps.append(m)
    return in_maps


def combine_outputs(outs):
    """outs: list of per-core [48, 1] arrays -> scalar float32."""
    s_tot = 0.0
    q_tot = 0.0
    for o in outs:
        o = np.asarray(o, dtype=np.float64).reshape(48)
        s_tot += o[0:16].sum()
        q_tot += o[16:48].sum()
    sldj = B * s_tot - 0.5 * q_tot - B * N * 0.5 * np.log(2.0 * np.pi)
    return np.array(-sldj, dtype=np.float32)


def kernel(**inputs):
    from concourse.bass_utils import run_bass_kernel_spmd

    nc = get_program()
    in_maps = make_in_maps(inputs)
    res = run_bass_kernel_spmd(nc, in_maps, core_ids=list(range(NCORES)))
    return combine_outputs([r["out"] for r in res.results])


# revision 25
# speedup vs baseline: 1.6159x; 1.2640x over previous
"""Trainium2 Bass kernel for nn_PixelTransformer.

Math notes (derived from the reference semantics, valid for ANY input values):
  * The transformer hidden state is built purely from positional encodings
    (x never enters it), broadcast over the batch.  The attention mixes only
    across the batch axis (head_dim=1), so with identical tokens per batch the
    softmax is uniform and the attention output equals v exactly.  Attention +
    residual therefore folds into a per-layer 5x5 linear map.
  * LayerNorm centering is a linear projection C = I - J/5, foldable into the
    preceding matmuls; LN affine params fold into the following matmuls.
  * LayerNorm is invariant to per-pixel positive scaling, and ReLU commutes
    with it.  The kernel therefore keeps the state in an UNSCALED
    representation u with g_true = (1/std) * u, tracking std per pixel.
    Biases become rank-1 accumulating matmuls (bias_row x std), and the eps
    of 1/sqrt(var+eps) becomes a rank-1 term (eps x std_prev^2) in the
    variance matmul.  Each LN is then just Square -> matmul -> Sqrt; no
    normalization is ever materialized.
  * The 16-step affine flow scan has the closed form
      z = exp(S) * x + sum_j exp(sum_{k>j} sc_k) * t_j
    computed with one triangular-matrix matmul.
  * Output is a scalar; each core emits 48 partial sums, host combines.

Sharding: the N=1024 pixels are split across 8 cores (128 each); all weights
are replicated.  Device layout keeps features on partitions and pixels on the
free dimension, so no transposes are ever needed.  The FFN (5->2048->5) and
the rank-1 bias terms run in bf16 (fp32 accumulation); the main residual
path stays fp32.
"""

import numpy as np

B, H, W = 32, 32, 32
N = H * W
L, D, FF = 8, 5, 2048
NCORES = 8
NP = N // NCORES          # pixels per core
NCHUNK = FF // 128        # 16 ff chunks of 128
NQ = NCHUNK // 4          # 4 chunks batched per PSUM bank
EPS = 1e-5

_PROG = None              # cached compiled Bass program


def _build_program():
    import concourse.bacc as bacc
    import concourse.mybir as mybir
    import concourse.tile as tile

    f32 = mybir.dt.float32
    bf16 = mybir.dt.bfloat16
    AF = mybir.ActivationFunctionType
    ALU = mybir.AluOpType
    AX = mybir.AxisListType

    nc = bacc.Bacc(name="pixel_transformer")

    smalls_d = nc.dram_tensor("smalls", [5, 80 + NP], f32, kind="ExternalInput")
    brows_d = nc.dram_tensor("brows", [1, 81], bf16, kind="ExternalInput")
    hrows_d = nc.dram_tensor("hrows", [1, 48], f32, kind="ExternalInput")
    smallsb_d = nc.dram_tensor("smallsb", [5, 40], bf16, kind="ExternalInput")
    w1b_d = [
        nc.dram_tensor(f"w1b{l}", [5, FF], bf16, kind="ExternalInput")
        for l in range(L)
    ]
    b1b_d = [
        nc.dram_tensor(f"b1b{l}", [1, FF], bf16, kind="ExternalInput")
        for l in range(L)
    ]
    w2b_d = nc.dram_tensor("w2b", [128, L * NCHUNK * D], bf16, kind="ExternalInput")
    headw_d = nc.dram_tensor("headw", [16, 66], f32, kind="ExternalInput")
    xsh_d = nc.dram_tensor("xsh", [B, NP], f32, kind="ExternalInput")
    out_d = nc.dram_tensor("out", [48, 1], f32, kind="ExternalOutput")
    outz_d = nc.dram_tensor("outz", [B, NP], f32, kind="ExternalOutput")

    with tile.TileContext(nc) as tc:
        with (
            tc.tile_pool(name="consts", bufs=1) as cp,
            tc.tile_pool(name="work", bufs=2) as wp,
            tc.tile_pool(name="fsb", bufs=3) as fp,
            tc.tile_pool(name="ps", bufs=2, space="PSUM") as pp,
        ):
            # --- input DMAs ---
            # SP: layer-0 weights + small critical tensors first, then the
            # per-layer w1 / b1-low-half stream.  Pool: w2 + all b1 high
            # halves (early).  ACT: b1-low of layer 0 only; head tensors are
            # DMA'd mid-program.
            HFF = FF // 2
            w1sb = []
            b1lo = []
            b1hi = []
            for l in range(L):
                w1sb.append(cp.tile([5, FF], bf16, name=f"w1sb{l}"))
                b1lo.append(cp.tile([1, HFF], bf16, name=f"b1lo{l}"))
                b1hi.append(cp.tile([1, HFF], bf16, name=f"b1hi{l}"))
            smalls = cp.tile([5, 80 + NP], f32)
            nc.sync.dma_start(out=smalls, in_=smalls_d[:, :])
            brows = cp.tile([1, 81], bf16)
            nc.sync.dma_start(out=brows, in_=brows_d[:, :])
            nc.sync.dma_start(out=w1sb[0], in_=w1b_d[0][:, :])
            toksb = smalls[:, 80:80 + NP]
            for l in range(1, L):
                nc.sync.dma_start(out=b1lo[l], in_=b1b_d[l][:, 0:HFF])
                nc.sync.dma_start(out=w1sb[l], in_=w1b_d[l][:, :])
            smallsb = cp.tile([5, 40], bf16)
            nc.gpsimd.dma_start(out=smallsb, in_=smallsb_d[:, :])
            nc.gpsimd.dma_start(out=b1lo[0], in_=b1b_d[0][:, 0:HFF])
            nc.gpsimd.dma_start(out=b1hi[0], in_=b1b_d[0][:, HFF:FF])
            w2sb = cp.tile([128, L * NCHUNK * D], bf16)
            nc.gpsimd.dma_start(out=w2sb, in_=w2b_d[:, :])
            nc.gpsimd.dma_start(out=b1hi[1], in_=b1b_d[1][:, HFF:FF])
            headsb = cp.tile([16, 66], f32)
            nc.gpsimd.dma_start(out=headsb, in_=headw_d[:, :])
            hrows = cp.tile([1, 48], f32)
            nc.gpsimd.dma_start(out=hrows, in_=hrows_d[:, :])
            xsb = cp.tile([B, NP], f32)
            nc.gpsimd.dma_start(out=xsb, in_=xsh_d[:, :])
            for l in range(2, L):
                nc.gpsimd.dma_start(out=b1hi[l], in_=b1b_d[l][:, HFF:FF])

            # --- constants via memset ---
            vconst = cp.tile([5, 1], bf16)      # 1/D rows for variance matmul
            nc.vector.memset(vconst, 1.0 / D)
            vconstf = cp.tile([1, 1], f32)
            nc.vector.memset(vconstf, 1.0)
            ones16 = cp.tile([16, 1], f32)      # column-sum over 16 partitions
            nc.vector.memset(ones16, 1.0)
            ones_b16 = cp.tile([1, 16], f32)    # broadcast 1 -> 16 partitions
            nc.vector.memset(ones_b16, 1.0)
            ones_b32 = cp.tile([1, B], f32)     # broadcast 1 -> 32 partitions
            nc.vector.memset(ones_b32, 1.0)
            ones16x32 = cp.tile([16, B], f32)   # col-sum 16 + broadcast -> 32
            nc.vector.memset(ones16x32, 1.0)
            std0b = cp.tile([1, NP], bf16)      # std before layer 0 == 1
            nc.vector.memset(std0b, 1.0)
            psv0b = cp.tile([1, NP], bf16)      # var+eps proxy before layer 0
            nc.vector.memset(psv0b, 1.0)
            from bass_rust import add_dep_helper
            warmt = cp.tile([1, 1], f32)
            warm_insts = [
                nc.scalar.activation(out=warmt, in_=vconstf[0:1, 0:1], func=f)
                for f in (AF.Square, AF.Sqrt, AF.Relu)
            ]

            def ln_v2(ps_y, psvb_prev, idx, fp32_std=False,
                      want_psvb=True, want_u=True):
                """LN in the scaled representation: unscaled centered PSUM
                ps_y -> (u [5,NP] f32, std bf16 (or f32), psv bf16)."""
                sq = wp.tile([5, NP], bf16, tag="sq", name=f"sq{idx}")
                nc.scalar.activation(out=sq, in_=ps_y, func=AF.Square)
                psv = pp.tile([1, NP], f32, tag="psmall", name=f"psv{idx}")
                nc.tensor.matmul(
                    psv, brows[:, 80:81], psvb_prev, start=True, stop=False
                )
                nc.tensor.matmul(psv, vconst, sq, start=False, stop=True)
                u = None
                if want_u:
                    u = wp.tile([5, NP], f32, tag="u", bufs=3, name=f"u{idx}")
                    nc.vector.tensor_copy(out=u, in_=ps_y)
                stdt = f32 if fp32_std else bf16
                stdb = wp.tile([1, NP], stdt, tag="stdb", bufs=3,
                               name=f"stdb{idx}")
                ln_v2.last_sqrt = nc.scalar.activation(
                    out=stdb, in_=psv, func=AF.Sqrt
                )
                psvb = None
                if want_psvb:
                    psvb = wp.tile([1, NP], bf16, tag="psvb", bufs=3,
                                   name=f"psvb{idx}")
                    nc.vector.tensor_copy(out=psvb, in_=psv)
                return u, stdb, psvb

            u_prev, stdb_prev, psvb_prev = toksb, std0b, psv0b
            for l in range(L):
                # attention (folded) + residual + LN1 centering
                psy = pp.tile([D, NP], f32, tag="psmall", name=f"psy{l}")
                ma_inst = nc.tensor.matmul(
                    psy, smalls[:, 5 * l:5 * l + 5], u_prev,
                    start=True, stop=False,
                )
                if l == 0:
                    for wi in warm_insts:
                        add_dep_helper(ma_inst.ins, wi.ins,
                                       reason="act table warm before layer 0")
                nc.tensor.matmul(
                    psy, brows[:, 5 * l:5 * l + 5], stdb_prev,
                    start=False, stop=True,
                )
                u1b = wp.tile([5, NP], bf16, tag="ub", name=f"ub{l}")
                nc.vector.tensor_copy(out=u1b, in_=psy)
                _, std1b, psvb1 = ln_v2(psy, psvb_prev, f"a{l}", want_u=False)

                # FFN in bf16; psum_y2 accumulates the centered layer output
                psy2 = pp.tile([D, NP], f32, tag="pacc", bufs=2, name=f"psy2{l}")
                nc.tensor.matmul(
                    psy2, smallsb[:, 5 * l:5 * l + 5], u1b,
                    start=True, stop=False,
                )
                # all mm1 mains first (only need u1b, ready early); the
                # rank-1 bias matmuls wait on sqrt -> keep them behind so
                # they don't block the mains in PE program order
                psfs = []
                for q in range(NQ):
                    psf = pp.tile([128, 512], f32, tag="pf", bufs=4,
                                  name=f"psf{l}_{q}")
                    psfs.append(psf)
                    for c4 in range(4):
                        c = 4 * q + c4
                        nc.tensor.matmul(
                            psf[:, 128 * c4:128 * (c4 + 1)],
                            w1sb[l][:, 128 * c:128 * (c + 1)],
                            u1b, start=(c4 == 0), stop=False,
                        )
                for q in range(NQ):
                    for c4 in range(4):
                        c = 4 * q + c4
                        bsrc = b1lo[l] if c < 8 else b1hi[l]
                        cc = c % 8
                        nc.tensor.matmul(
                            psfs[q][:, 128 * c4:128 * (c4 + 1)],
                            bsrc[:, 128 * cc:128 * (cc + 1)],
                            std1b, start=False, stop=(c4 == 3),
                        )
                nc.tensor.matmul(
                    psy2, brows[:, 40 + 5 * l:45 + 5 * l], std1b,
                    start=False, stop=False,
                )
                for q in range(NQ):
                    fq = fp.tile([128, 512], bf16, tag="f", name=f"f{l}_{q}")
                    if q % 2 == 0:
                        nc.scalar.activation(out=fq, in_=psfs[q], func=AF.Relu)
                    else:
                        nc.vector.tensor_scalar(
                            out=fq, in0=psfs[q], scalar1=0.0, scalar2=None,
                            op0=ALU.max,
                        )
                    for c4 in range(4):
                        c = 4 * q + c4
                        nc.tensor.matmul(
                            psy2,
                            w2sb[:, (NCHUNK * l + c) * D:(NCHUNK * l + c + 1) * D],
                            fq[:, 128 * c4:128 * (c4 + 1)],
                            start=False,
                            stop=(c == NCHUNK - 1),
                        )
                u_prev, stdb_prev, psvb_prev = ln_v2(
                    psy2, psvb1, f"b{l}", fp32_std=(l == L - 1),
                    want_psvb=(l < L - 1),
                )
            std8 = stdb_prev
            last_ln_inst = ln_v2.last_sqrt

            # ---- head ----
            # hid_u = relu(f0w1'@u8 + f0b1' x std8)
            psh = pp.tile([16, NP], f32, tag="psmall")
            nc.tensor.matmul(psh, hrows[:, 0:16], std8, start=True, stop=False)
            nc.tensor.matmul(psh, headsb[0:5, 0:16], u_prev, start=False, stop=True)
            hid = wp.tile([16, NP], f32, tag="sq")
            nc.vector.tensor_scalar(
                out=hid, in0=psh, scalar1=0.0, scalar2=None, op0=ALU.max
            )

            # st halves, still scaled by 1/std8
            psss = pp.tile([16, NP], f32, tag="psmall")
            nc.tensor.matmul(psss, hrows[:, 16:32], std8, start=True, stop=False)
            nc.tensor.matmul(psss, headsb[:, 16:32], hid, start=False, stop=True)
            psst = pp.tile([16, NP], f32, tag="pf", bufs=4)
            nc.tensor.matmul(psst, hrows[:, 32:48], std8, start=True, stop=False)
            nc.tensor.matmul(psst, headsb[:, 32:48], hid, start=False, stop=True)

            # materialize absolute st: r8 = 1/std8 broadcast to 16 partitions
            rec8 = wp.tile([1, NP], f32, tag="rec8")
            nc.vector.reciprocal(out=rec8, in_=std8)
            psbc = pp.tile([16, NP], f32, tag="pacc", bufs=2)
            nc.tensor.matmul(psbc, ones_b16, rec8, start=True, stop=True)
            r8bc = wp.tile([16, NP], f32, tag="r8bc")
            nc.vector.tensor_copy(out=r8bc, in_=psbc)
            s_sb = wp.tile([16, NP], f32, tag="u", bufs=3)
            nc.vector.tensor_mul(out=s_sb, in0=psss, in1=r8bc)

            sf = wp.tile([16, 1], f32, tag="sf")
            sf_inst = nc.scalar.activation(
                out=sf, in_=headsb[0:16, 48:49], func=AF.Exp
            )
            add_dep_helper(sf_inst.ins, last_ln_inst.ins,
                           reason="keep head Exp behind layer ACT ops")
            rsf = wp.tile([16, 1], f32, tag="rsf")
            nc.vector.reciprocal(out=rsf, in_=sf)
            sc1 = wp.tile([16, NP], f32, tag="sc1")
            nc.vector.tensor_scalar(
                out=sc1, in0=s_sb, scalar1=rsf, scalar2=None, op0=ALU.mult
            )
            th = wp.tile([16, NP], f32, tag="th")
            nc.scalar.activation(out=th, in_=sc1, func=AF.Tanh)
            sc = wp.tile([16, NP], f32, tag="sc")
            nc.vector.tensor_scalar(
                out=sc, in0=th, scalar1=sf, scalar2=None, op0=ALU.mult
            )
            t_sb = wp.tile([16, NP], f32, tag="ub")
            nc.vector.tensor_mul(out=t_sb, in0=psst, in1=r8bc)
            ssum = wp.tile([16, 1], f32, tag="ssum")
            nc.vector.reduce_sum(out=ssum, in_=s_sb, axis=AX.X)
            nc.sync.dma_start(out=out_d[0:16, :], in_=ssum)

            # flow closed form
            psD = pp.tile([16, NP], f32, tag="psmall")
            nc.tensor.matmul(psD, headsb[0:16, 49:65], sc, start=True, stop=True)
            psS = pp.tile([1, NP], f32, tag="pf", bufs=4)
            nc.tensor.matmul(psS, ones16, sc, start=True, stop=True)
            wexp = wp.tile([16, NP], f32, tag="sc1")
            nc.scalar.activation(out=wexp, in_=psD, func=AF.Exp)
            wt = wp.tile([16, NP], f32, tag="th")
            nc.vector.tensor_mul(out=wt, in0=wexp, in1=t_sb)
            pscb = pp.tile([B, NP], f32, tag="psmall")
            nc.tensor.matmul(pscb, ones16x32, wt, start=True, stop=True)

            eS = wp.tile([1, NP], f32, tag="eS")
            nc.scalar.activation(out=eS, in_=psS, func=AF.Exp)
            pseb = pp.tile([B, NP], f32, tag="pacc", bufs=2)
            nc.tensor.matmul(pseb, ones_b32, eS, start=True, stop=True)
            zt = wp.tile([B, NP], f32, tag="zt")
            nc.vector.tensor_mul(out=zt, in0=xsb, in1=pseb)

            z = wp.tile([B, NP], f32, tag="z")
            nc.vector.tensor_add(out=z, in0=zt, in1=pscb)
            zsq = wp.tile([B, NP], f32, tag="zt2")
            nc.scalar.activation(out=zsq, in_=z, func=AF.Square)
            nc.sync.dma_start(out=outz_d[:, :], in_=zsq)

    nc.finalize()
    return nc


def _fold_inputs(inp):
    """Host-side weight folding (float64 for precision, cast at the end)."""
    import ml_dtypes

    C = np.eye(D) - np.ones((D, D)) / D
    g = lambda k: np.asarray(inp[k], dtype=np.float64)
    wqkv, bqkv, wo, bo = g("wqkv"), g("bqkv"), g("wo"), g("bo")
    w1, b1, w2, b2 = g("w1"), g("b1"), g("w2"), g("b2")
    ln1w, ln1b, ln2w, ln2b = g("ln1w"), g("ln1b"), g("ln2w"), g("ln2b")

    smalls = np.zeros((5, 80), np.float64)
    brows = np.zeros((1, 81), np.float64)
    w1a = []
    b1b = np.zeros((L, 1, FF), np.float64)
    w2b = np.zeros((128, L * NCHUNK * D), np.float64)
    for l in range(L):
        Dl = np.diag(ln2w[l - 1]) if l > 0 else np.eye(D)
        el = ln2b[l - 1] if l > 0 else np.zeros(D)
        wv = wqkv[l][2 * D:3 * D, :]
        bv = bqkv[l][2 * D:3 * D]
        A0 = np.eye(D) + wo[l] @ wv
        c_attn = wo[l] @ bv + bo[l]
        smalls[:, 5 * l:5 * l + 5] = (C @ A0 @ Dl).T
        brows[0, 5 * l:5 * l + 5] = C @ (A0 @ el + c_attn)
        smalls[:, 40 + 5 * l:45 + 5 * l] = (C @ np.diag(ln1w[l])).T
        brows[0, 40 + 5 * l:45 + 5 * l] = C @ (ln1b[l] + b2[l])
        w1a.append((w1[l] * ln1w[l][None, :]).T)       # [5, FF]
        b1b[l, 0, :] = b1[l] + w1[l] @ ln1b[l]
        w2full = (C @ w2[l]).T                         # [2048, 5]
        for c in range(NCHUNK):
            w2b[:, (NCHUNK * l + c) * D:(NCHUNK * l + c + 1) * D] = (
                w2full[128 * c:128 * (c + 1), :]
            )
    brows[0, 80] = EPS

    f0w1, f0b1 = g("f0w1"), g("f0b1")
    f0w2, f0b2 = g("f0w2"), g("f0b2")
    D8 = np.diag(ln2w[L - 1])
    e8 = ln2b[L - 1]
    headw = np.zeros((16, 66), np.float64)
    headw[0:5, 0:16] = (f0w1 @ D8).T
    headw[0:16, 16:32] = f0w2.T[:, 0:16]
    headw[0:16, 32:48] = f0w2.T[:, 16:32]
    headw[0:16, 48] = float(np.asarray(inp["sfac"])[0])
    for j in range(16):
        headw[j + 1:16, 49 + j] = 1.0                 # sum_{k>j}
    hrows = np.zeros((1, 48), np.float64)
    hrows[0, 0:16] = f0b1 + f0w1 @ e8
    hrows[0, 16:32] = f0b2[0:16]
    hrows[0, 32:48] = f0b2[16:32]

    # positional tokens, exactly as the reference builds them (fp32 ops)
    xs = (np.arange(W, dtype=np.float32) / np.float32(1e4)).astype(np.float32)
    ys = (np.arange(H, dtype=np.float32) / np.float32(1e4)).astype(np.float32)
    sinx = np.broadcast_to(np.sin(xs)[None, :], (H, W)).reshape(N)
    cosx = np.broadcast_to(np.cos(xs)[None, :], (H, W)).reshape(N)
    siny = np.broadcast_to(np.sin(ys)[:, None], (H, W)).reshape(N)
    cosy = np.broadcast_to(np.cos(ys)[:, None], (H, W)).reshape(N)
    tok = np.stack(
        [-np.ones(N, np.float32), sinx, cosx, siny, cosy], axis=0
    )                                                  # [5, N]
    xflat = np.asarray(inp["x"], dtype=np.float32)[:, 0].reshape(B, N)

    out = {
        "smalls": np.concatenate(
            [smalls, tok.astype(np.float64)], axis=1
        ).astype(np.float32),
        "smallsb": smalls[:, 40:80].astype(ml_dtypes.bfloat16),
        "brows": brows.astype(ml_dtypes.bfloat16),
        "hrows": hrows.astype(np.float32),
        "w2b": w2b.astype(ml_dtypes.bfloat16),
        "headw": headw.astype(np.float32),
        "tok": tok.astype(np.float32),
        "xsh": xflat,
    }
    for l in range(L):
        out[f"w1b{l}"] = w1a[l].astype(ml_dtypes.bfloat16)
        out[f"b1b{l}"] = b1b[l].astype(ml_dtypes.bfloat16)
    return out


def get_program():
    global _PROG
    if _PROG is None:
        _PROG = _build_program()
    return _PROG


def make_in_maps(inputs):
    arrs = _fold_inputs(inputs)
    shared_keys = (
        ["smallsb", "brows", "hrows", "w2b", "headw"]
        + [f"w1b{l}" for l in range(L)]
        + [f"b1b{l}" for l in range(L)]
    )
    shared = {k: arrs[k] for k in shared_keys}
    base = arrs["smalls"][:, 0:80]
    in_maps = []
    for core in range(NCORES):
        sl = slice(core * NP, (core + 1) * NP)
        m = dict(shared)
        m["smalls"] = np.ascontiguousarray(
            np.concatenate([base, arrs["tok"][:, sl]], axis=1)
        )
        m["xsh"] = np.ascontiguousarray(arrs["xsh"][:, sl])
        in_maps.append(m)
    return in_maps


def combine_outputs(outs, outzs):
    """per-core s-sums [48,1] and z^2 tiles [B,NP] -> scalar float32."""
    s_tot = 0.0
    q_tot = 0.0
    for o, oz in zip(outs, outzs):
        o = np.asarray(o, dtype=np.float64).reshape(48)
        s_tot += o[0:16].sum()
        q_tot += np.asarray(oz, dtype=np.float64).sum()
    sldj = B * s_tot - 0.5 * q_tot - B * N * 0.5 * np.log(2.0 * np.pi)
    return np.array(-sldj, dtype=np.float32)


def kernel(**inputs):
    from concourse.bass_utils import run_bass_kernel_spmd

    nc = get_program()
    in_maps = make_in_maps(inputs)
    res = run_bass_kernel_spmd(nc, in_maps, core_ids=list(range(NCORES)))
    return combine_outputs([r["out"] for r in res.results],
                           [r["outz"] for r in res.results])


# revision 29
# speedup vs baseline: 1.6413x; 1.0157x over previous
"""Trainium2 Bass kernel for nn_PixelTransformer.

Math notes (derived from the reference semantics, valid for ANY input values):
  * The transformer hidden state is built purely from positional encodings
    (x never enters it), broadcast over the batch.  The attention mixes only
    across the batch axis (head_dim=1), so with identical tokens per batch the
    softmax is uniform and the attention output equals v exactly.  Attention +
    residual therefore folds into a per-layer 5x5 linear map.
  * LayerNorm centering is a linear projection C = I - J/5, foldable into the
    preceding matmuls; LN affine params fold into the following matmuls.
  * LayerNorm is invariant to per-pixel positive scaling, and ReLU commutes
    with it.  The kernel therefore keeps the state in an UNSCALED
    representation u with g_true = (1/std) * u, tracking std per pixel.
    Biases become rank-1 accumulating matmuls (bias_row x std), and the eps
    of 1/sqrt(var+eps) becomes a rank-1 term (eps x std_prev^2) in the
    variance matmul.  Each LN is then just Square -> matmul -> Sqrt; no
    normalization is ever materialized.
  * The 16-step affine flow scan has the closed form
      z = exp(S) * x + sum_j exp(sum_{k>j} sc_k) * t_j
    computed with one triangular-matrix matmul.
  * Output is a scalar; each core emits 48 partial sums, host combines.

Sharding: the N=1024 pixels are split across 8 cores (128 each); all weights
are replicated.  Device layout keeps features on partitions and pixels on the
free dimension, so no transposes are ever needed.  The FFN (5->2048->5) and
the rank-1 bias terms run in bf16 (fp32 accumulation); the main residual
path stays fp32.
"""

import numpy as np

B, H, W = 32, 32, 32
N = H * W
L, D, FF = 8, 5, 2048
NCORES = 8
NP = N // NCORES          # pixels per core
NCHUNK = FF // 128        # 16 ff chunks of 128
NQ = NCHUNK // 4          # 4 chunks batched per PSUM bank
EPS = 1e-5

_PROG = None              # cached compiled Bass program


def _build_program():
    import concourse.bacc as bacc
    import concourse.mybir as mybir
    import concourse.tile as tile

    f32 = mybir.dt.float32
    bf16 = mybir.dt.bfloat16
    AF = mybir.ActivationFunctionType
    ALU = mybir.AluOpType
    AX = mybir.AxisListType

    nc = bacc.Bacc(name="pixel_transformer")

    smalls_d = nc.dram_tensor("smalls", [5, 80 + NP], f32, kind="ExternalInput")
    brows_d = nc.dram_tensor("brows", [1, 81], bf16, kind="ExternalInput")
    hrows_d = nc.dram_tensor("hrows", [1, 48], f32, kind="ExternalInput")
    smallsb_d = nc.dram_tensor("smallsb", [5, 40], bf16, kind="ExternalInput")
    w1b_d = [
        nc.dram_tensor(f"w1b{l}", [5, FF], bf16, kind="ExternalInput")
        for l in range(L)
    ]
    b1b_d = [
        nc.dram_tensor(f"b1b{l}", [1, FF], bf16, kind="ExternalInput")
        for l in range(L)
    ]
    w2b_d = nc.dram_tensor("w2b", [128, L * NCHUNK * D], bf16, kind="ExternalInput")
    headw_d = nc.dram_tensor("headw", [16, 66], f32, kind="ExternalInput")
    xsh_d = nc.dram_tensor("xsh", [B, NP], f32, kind="ExternalInput")
    out_d = nc.dram_tensor("out", [48, 1], f32, kind="ExternalOutput")
    outz_d = nc.dram_tensor("outz", [B, NP], f32, kind="ExternalOutput")

    with tile.TileContext(nc) as tc:
        with (
            tc.tile_pool(name="consts", bufs=1) as cp,
            tc.tile_pool(name="work", bufs=2) as wp,
            tc.tile_pool(name="fsb", bufs=4) as fp,
            tc.tile_pool(name="ps", bufs=2, space="PSUM") as pp,
        ):
            # --- input DMAs ---
            # SP: layer-0 weights + small critical tensors first, then the
            # per-layer w1 / b1-low-half stream.  Pool: w2 + all b1 high
            # halves (early).  ACT: b1-low of layer 0 only; head tensors are
            # DMA'd mid-program.
            HFF = FF // 2
            w1sb = []
            b1lo = []
            b1hi = []
            for l in range(L):
                w1sb.append(cp.tile([5, FF], bf16, name=f"w1sb{l}"))
                b1lo.append(cp.tile([1, HFF], bf16, name=f"b1lo{l}"))
                b1hi.append(cp.tile([1, HFF], bf16, name=f"b1hi{l}"))
            smalls = cp.tile([5, 80 + NP], f32)
            nc.sync.dma_start(out=smalls, in_=smalls_d[:, :])
            brows = cp.tile([1, 81], bf16)
            nc.sync.dma_start(out=brows, in_=brows_d[:, :])
            nc.sync.dma_start(out=w1sb[0], in_=w1b_d[0][:, :])
            toksb = smalls[:, 80:80 + NP]
            for l in range(1, L):
                nc.sync.dma_start(out=b1lo[l], in_=b1b_d[l][:, 0:HFF])
                nc.sync.dma_start(out=w1sb[l], in_=w1b_d[l][:, :])
            smallsb = cp.tile([5, 40], bf16)
            nc.gpsimd.dma_start(out=smallsb, in_=smallsb_d[:, :])
            nc.gpsimd.dma_start(out=b1lo[0], in_=b1b_d[0][:, 0:HFF])
            nc.gpsimd.dma_start(out=b1hi[0], in_=b1b_d[0][:, HFF:FF])
            w2sb = cp.tile([128, L * NCHUNK * D], bf16)
            nc.gpsimd.dma_start(out=w2sb, in_=w2b_d[:, :])
            nc.gpsimd.dma_start(out=b1hi[1], in_=b1b_d[1][:, HFF:FF])
            headsb = cp.tile([16, 66], f32)
            nc.gpsimd.dma_start(out=headsb, in_=headw_d[:, :])
            hrows = cp.tile([1, 48], f32)
            nc.gpsimd.dma_start(out=hrows, in_=hrows_d[:, :])
            xsb = cp.tile([B, NP], f32)
            nc.gpsimd.dma_start(out=xsb, in_=xsh_d[:, :])
            for l in range(2, L):
                nc.gpsimd.dma_start(out=b1hi[l], in_=b1b_d[l][:, HFF:FF])

            # --- constants via memset ---
            vconst = cp.tile([5, 1], bf16)      # 1/D rows for variance matmul
            nc.vector.memset(vconst, 1.0 / D)
            vconstf = cp.tile([1, 1], f32)
            nc.vector.memset(vconstf, 1.0)
            ones16 = cp.tile([16, 1], f32)      # column-sum over 16 partitions
            nc.vector.memset(ones16, 1.0)
            ones_b16 = cp.tile([1, 16], f32)    # broadcast 1 -> 16 partitions
            nc.vector.memset(ones_b16, 1.0)
            ones_b32 = cp.tile([1, B], f32)     # broadcast 1 -> 32 partitions
            nc.vector.memset(ones_b32, 1.0)
            ones16x32 = cp.tile([16, B], f32)   # col-sum 16 + broadcast -> 32
            nc.vector.memset(ones16x32, 1.0)
            std0b = cp.tile([1, NP], bf16)      # std before layer 0 == 1
            nc.vector.memset(std0b, 1.0)
            psv0b = cp.tile([1, NP], bf16)      # var+eps proxy before layer 0
            nc.vector.memset(psv0b, 1.0)
            from bass_rust import add_dep_helper
            warmt = cp.tile([1, 1], f32)
            warm_insts = [
                nc.scalar.activation(out=warmt, in_=vconstf[0:1, 0:1], func=f)
                for f in (AF.Square, AF.Sqrt, AF.Relu)
            ]

            def ln_v2(ps_y, psvb_prev, idx, fp32_std=False,
                      want_psvb=True, want_u=True):
                """LN in the scaled representation: unscaled centered PSUM
                ps_y -> (u [5,NP] f32, std bf16 (or f32), psv bf16)."""
                sq = wp.tile([5, NP], bf16, tag="sq", name=f"sq{idx}")
                nc.scalar.activation(out=sq, in_=ps_y, func=AF.Square)
                psv = pp.tile([1, NP], f32, tag="psmall", name=f"psv{idx}")
                nc.tensor.matmul(
                    psv, brows[:, 80:81], psvb_prev, start=True, stop=False
                )
                nc.tensor.matmul(psv, vconst, sq, start=False, stop=True)
                u = None
                if want_u:
                    u = wp.tile([5, NP], f32, tag="u", bufs=3, name=f"u{idx}")
                    nc.vector.tensor_copy(out=u, in_=ps_y)
                stdt = f32 if fp32_std else bf16
                stdb = wp.tile([1, NP], stdt, tag="stdb", bufs=3,
                               name=f"stdb{idx}")
                ln_v2.last_sqrt = nc.scalar.activation(
                    out=stdb, in_=psv, func=AF.Sqrt
                )
                psvb = None
                if want_psvb:
                    psvb = wp.tile([1, NP], bf16, tag="psvb", bufs=3,
                                   name=f"psvb{idx}")
                    nc.vector.tensor_copy(out=psvb, in_=psv)
                return u, stdb, psvb

            u_prev, stdb_prev, psvb_prev = toksb, std0b, psv0b
            for l in range(L):
                # attention (folded) + residual + LN1 centering
                psy = pp.tile([D, NP], f32, tag="psmall", name=f"psy{l}")
                ma_inst = nc.tensor.matmul(
                    psy, smalls[:, 5 * l:5 * l + 5], u_prev,
                    start=True, stop=False,
                )
                if l == 0:
                    for wi in warm_insts:
                        add_dep_helper(ma_inst.ins, wi.ins,
                                       reason="act table warm before layer 0")
                nc.tensor.matmul(
                    psy, brows[:, 5 * l:5 * l + 5], stdb_prev,
                    start=False, stop=True,
                )
                u1b = wp.tile([5, NP], bf16, tag="ub", name=f"ub{l}")
                nc.vector.tensor_copy(out=u1b, in_=psy)
                _, std1b, psvb1 = ln_v2(psy, psvb_prev, f"a{l}", want_u=False)

                # FFN in bf16; psum_y2 accumulates the centered layer output
                psy2 = pp.tile([D, NP], f32, tag="pacc", bufs=2, name=f"psy2{l}")
                nc.tensor.matmul(
                    psy2, smallsb[:, 5 * l:5 * l + 5], u1b,
                    start=True, stop=False,
                )
                # all mm1 mains first (only need u1b, ready early); the
                # rank-1 bias matmuls wait on sqrt -> keep them behind so
                # they don't block the mains in PE program order
                psfs = []
                for q in range(NQ):
                    psf = pp.tile([128, 512], f32, tag="pf", bufs=4,
                                  name=f"psf{l}_{q}")
                    psfs.append(psf)
                    for c4 in range(4):
                        c = 4 * q + c4
                        nc.tensor.matmul(
                            psf[:, 128 * c4:128 * (c4 + 1)],
                            w1sb[l][:, 128 * c:128 * (c + 1)],
                            u1b, start=(c4 == 0), stop=False,
                        )
                for q in range(NQ):
                    for c4 in range(4):
                        c = 4 * q + c4
                        bsrc = b1lo[l] if c < 8 else b1hi[l]
                        cc = c % 8
                        nc.tensor.matmul(
                            psfs[q][:, 128 * c4:128 * (c4 + 1)],
                            bsrc[:, 128 * cc:128 * (cc + 1)],
                            std1b, start=False, stop=(c4 == 3),
                        )
                nc.tensor.matmul(
                    psy2, brows[:, 40 + 5 * l:45 + 5 * l], std1b,
                    start=False, stop=False,
                )
                for q in range(NQ):
                    fq = fp.tile([128, 512], bf16, tag="f", name=f"f{l}_{q}")
                    if q % 2 == 1:
                        nc.scalar.activation(out=fq, in_=psfs[q], func=AF.Relu)
                    else:
                        nc.vector.tensor_scalar(
                            out=fq, in0=psfs[q], scalar1=0.0, scalar2=None,
                            op0=ALU.max,
                        )
                    for c4 in range(4):
                        c = 4 * q + c4
                        nc.tensor.matmul(
                            psy2,
                            w2sb[:, (NCHUNK * l + c) * D:(NCHUNK * l + c + 1) * D],
                            fq[:, 128 * c4:128 * (c4 + 1)],
                            start=False,
                            stop=(c == NCHUNK - 1),
                        )
                u_prev, stdb_prev, psvb_prev = ln_v2(
                    psy2, psvb1, f"b{l}", fp32_std=(l == L - 1),
                    want_psvb=(l < L - 1),
                )
            std8 = stdb_prev
            last_ln_inst = ln_v2.last_sqrt

            # ---- head ----
            # hid_u = relu(f0w1'@u8 + f0b1' x std8)
            psh = pp.tile([16, NP], f32, tag="psmall")
            nc.tensor.matmul(psh, hrows[:, 0:16], std8, start=True, stop=False)
            nc.tensor.matmul(psh, headsb[0:5, 0:16], u_prev, start=False, stop=True)
            hid = wp.tile([16, NP], f32, tag="sq")
            nc.vector.tensor_scalar(
                out=hid, in0=psh, scalar1=0.0, scalar2=None, op0=ALU.max
            )

            # st halves, still scaled by 1/std8
            psss = pp.tile([16, NP], f32, tag="psmall")
            nc.tensor.matmul(psss, hrows[:, 16:32], std8, start=True, stop=False)
            nc.tensor.matmul(psss, headsb[:, 16:32], hid, start=False, stop=True)
            psst = pp.tile([16, NP], f32, tag="pf", bufs=4)
            nc.tensor.matmul(psst, hrows[:, 32:48], std8, start=True, stop=False)
            nc.tensor.matmul(psst, headsb[:, 32:48], hid, start=False, stop=True)

            # materialize absolute st: r8 = 1/std8 broadcast to 16 partitions
            rec8 = wp.tile([1, NP], f32, tag="rec8")
            nc.vector.reciprocal(out=rec8, in_=std8)
            psbc = pp.tile([16, NP], f32, tag="pacc", bufs=2)
            nc.tensor.matmul(psbc, ones_b16, rec8, start=True, stop=True)
            r8bc = wp.tile([16, NP], f32, tag="r8bc")
            nc.vector.tensor_copy(out=r8bc, in_=psbc)
            s_sb = wp.tile([16, NP], f32, tag="u", bufs=3)
            nc.vector.tensor_mul(out=s_sb, in0=psss, in1=r8bc)

            sf = wp.tile([16, 1], f32, tag="sf")
            sf_inst = nc.scalar.activation(
                out=sf, in_=headsb[0:16, 48:49], func=AF.Exp
            )
            add_dep_helper(sf_inst.ins, last_ln_inst.ins,
                           reason="keep head Exp behind layer ACT ops")
            rsf = wp.tile([16, 1], f32, tag="rsf")
            nc.vector.reciprocal(out=rsf, in_=sf)
            sc1 = wp.tile([16, NP], f32, tag="sc1")
            nc.vector.tensor_scalar(
                out=sc1, in0=s_sb, scalar1=rsf, scalar2=None, op0=ALU.mult
            )
            th = wp.tile([16, NP], f32, tag="th")
            nc.scalar.activation(out=th, in_=sc1, func=AF.Tanh)
            sc = wp.tile([16, NP], f32, tag="sc")
            nc.vector.tensor_scalar(
                out=sc, in0=th, scalar1=sf, scalar2=None, op0=ALU.mult
            )
            t_sb = wp.tile([16, NP], f32, tag="ub")
            nc.vector.tensor_mul(out=t_sb, in0=psst, in1=r8bc)
            ssum = wp.tile([16, 1], f32, tag="ssum")
            nc.vector.reduce_sum(out=ssum, in_=s_sb, axis=AX.X)
            nc.sync.dma_start(out=out_d[0:16, :], in_=ssum)

            # flow closed form
            psD = pp.tile([16, NP], f32, tag="psmall")
            nc.tensor.matmul(psD, headsb[0:16, 49:65], sc, start=True, stop=True)
            psS = pp.tile([1, NP], f32, tag="pf", bufs=4)
            nc.tensor.matmul(psS, ones16, sc, start=True, stop=True)
            wexp = wp.tile([16, NP], f32, tag="sc1")
            nc.scalar.activation(out=wexp, in_=psD, func=AF.Exp)
            wt = wp.tile([16, NP], f32, tag="th")
            nc.vector.tensor_mul(out=wt, in0=wexp, in1=t_sb)
            pscb = pp.tile([B, NP], f32, tag="psmall")
            nc.tensor.matmul(pscb, ones16x32, wt, start=True, stop=True)

            eS = wp.tile([1, NP], f32, tag="eS")
            nc.scalar.activation(out=eS, in_=psS, func=AF.Exp)
            pseb = pp.tile([B, NP], f32, tag="pacc", bufs=2)
            nc.tensor.matmul(pseb, ones_b32, eS, start=True, stop=True)
            zt = wp.tile([B, NP], f32, tag="zt")
            nc.vector.tensor_mul(out=zt, in0=xsb, in1=pseb)

            z = wp.tile([B, NP], f32, tag="z")
            nc.vector.tensor_add(out=z, in0=zt, in1=pscb)
            zsq = wp.tile([B, NP], f32, tag="zt2")
            nc.scalar.activation(out=zsq, in_=z, func=AF.Square)
            nc.sync.dma_start(out=outz_d[:, :], in_=zsq)

    nc.finalize()
    return nc


def _fold_inputs(inp):
    """Host-side weight folding (float64 for precision, cast at the end)."""
    import ml_dtypes

    C = np.eye(D) - np.ones((D, D)) / D
    g = lambda k: np.asarray(inp[k], dtype=np.float64)
    wqkv, bqkv, wo, bo = g("wqkv"), g("bqkv"), g("wo"), g("bo")
    w1, b1, w2, b2 = g("w1"), g("b1"), g("w2"), g("b2")
    ln1w, ln1b, ln2w, ln2b = g("ln1w"), g("ln1b"), g("ln2w"), g("ln2b")

    smalls = np.zeros((5, 80), np.float64)
    brows = np.zeros((1, 81), np.float64)
    w1a = []
    b1b = np.zeros((L, 1, FF), np.float64)
    w2b = np.zeros((128, L * NCHUNK * D), np.float64)
    for l in range(L):
        Dl = np.diag(ln2w[l - 1]) if l > 0 else np.eye(D)
        el = ln2b[l - 1] if l > 0 else np.zeros(D)
        wv = wqkv[l][2 * D:3 * D, :]
        bv = bqkv[l][2 * D:3 * D]
        A0 = np.eye(D) + wo[l] @ wv
        c_attn = wo[l] @ bv + bo[l]
        smalls[:, 5 * l:5 * l + 5] = (C @ A0 @ Dl).T
        brows[0, 5 * l:5 * l + 5] = C @ (A0 @ el + c_attn)
        smalls[:, 40 + 5 * l:45 + 5 * l] = (C @ np.diag(ln1w[l])).T
        brows[0, 40 + 5 * l:45 + 5 * l] = C @ (ln1b[l] + b2[l])
        w1a.append((w1[l] * ln1w[l][None, :]).T)       # [5, FF]
        b1b[l, 0, :] = b1[l] + w1[l] @ ln1b[l]
        w2full = (C @ w2[l]).T                         # [2048, 5]
        for c in range(NCHUNK):
            w2b[:, (NCHUNK * l + c) * D:(NCHUNK * l + c + 1) * D] = (
                w2full[128 * c:128 * (c + 1), :]
            )
    brows[0, 80] = EPS

    f0w1, f0b1 = g("f0w1"), g("f0b1")
    f0w2, f0b2 = g("f0w2"), g("f0b2")
    D8 = np.diag(ln2w[L - 1])
    e8 = ln2b[L - 1]
    headw = np.zeros((16, 66), np.float64)
    headw[0:5, 0:16] = (f0w1 @ D8).T
    headw[0:16, 16:32] = f0w2.T[:, 0:16]
    headw[0:16, 32:48] = f0w2.T[:, 16:32]
    headw[0:16, 48] = float(np.asarray(inp["sfac"])[0])
    for j in range(16):
        headw[j + 1:16, 49 + j] = 1.0                 # sum_{k>j}
    hrows = np.zeros((1, 48), np.float64)
    hrows[0, 0:16] = f0b1 + f0w1 @ e8
    hrows[0, 16:32] = f0b2[0:16]
    hrows[0, 32:48] = f0b2[16:32]

    # positional tokens, exactly as the reference builds them (fp32 ops)
    xs = (np.arange(W, dtype=np.float32) / np.float32(1e4)).astype(np.float32)
    ys = (np.arange(H, dtype=np.float32) / np.float32(1e4)).astype(np.float32)
    sinx = np.broadcast_to(np.sin(xs)[None, :], (H, W)).reshape(N)
    cosx = np.broadcast_to(np.cos(xs)[None, :], (H, W)).reshape(N)
    siny = np.broadcast_to(np.sin(ys)[:, None], (H, W)).reshape(N)
    cosy = np.broadcast_to(np.cos(ys)[:, None], (H, W)).reshape(N)
    tok = np.stack(
        [-np.ones(N, np.float32), sinx, cosx, siny, cosy], axis=0
    )                                                  # [5, N]
    xflat = np.asarray(inp["x"], dtype=np.float32)[:, 0].reshape(B, N)

    out = {
        "smalls": np.concatenate(
            [smalls, tok.astype(np.float64)], axis=1
        ).astype(np.float32),
        "smallsb": smalls[:, 40:80].astype(ml_dtypes.bfloat16),
        "brows": brows.astype(ml_dtypes.bfloat16),
        "hrows": hrows.astype(np.float32),
        "w2b": w2b.astype(ml_dtypes.bfloat16),
        "headw": headw.astype(np.float32),
        "tok": tok.astype(np.float32),
        "xsh": xflat,
    }
    for l in range(L):
        out[f"w1b{l}"] = w1a[l].astype(ml_dtypes.bfloat16)
        out[f"b1b{l}"] = b1b[l].astype(ml_dtypes.bfloat16)
    return out


def get_program():
    global _PROG
    if _PROG is None:
        _PROG = _build_program()
    return _PROG


def make_in_maps(inputs):
    arrs = _fold_inputs(inputs)
    shared_keys = (
        ["smallsb", "brows", "hrows", "w2b", "headw"]
        + [f"w1b{l}" for l in range(L)]
        + [f"b1b{l}" for l in range(L)]
    )
    shared = {k: arrs[k] for k in shared_keys}
    base = arrs["smalls"][:, 0:80]
    in_maps = []
    for core in range(NCORES):
        sl = slice(core * NP, (core + 1) * NP)
        m = dict(shared)
        m["smalls"] = np.ascontiguousarray(
            np.concatenate([base, arrs["tok"][:, sl]], axis=1)
        )
        m["xsh"] = np.ascontiguousarray(arrs["xsh"][:, sl])
        in_maps.append(m)
    return in_maps


def combine_outputs(outs, outzs):
    """per-core s-sums [48,1] and z^2 tiles [B,NP] -> scalar float32."""
    s_tot = 0.0
    q_tot = 0.0
    for o, oz in zip(outs, outzs):
        o = np.asarray(o, dtype=np.float64).reshape(48)
        s_tot += o[0:16].sum()
        q_tot += np.asarray(oz, dtype=np.float64).sum()
    sldj = B * s_tot - 0.5 * q_tot - B * N * 0.5 * np.log(2.0 * np.pi)
    return np.array(-sldj, dtype=np.float32)


def kernel(**inputs):
    from concourse.bass_utils import run_bass_kernel_spmd

    nc = get_program()
    in_maps = make_in_maps(inputs)
    res = run_bass_kernel_spmd(nc, in_maps, core_ids=list(range(NCORES)))
    return combine_outputs([r["out"] for r in res.results],
                           [r["outz"] for r in res.results])
